# revision 14
# baseline (speedup 1.0000x reference)
"""FP8 semi-sparse activation linear kernel for Trainium2 (8 NeuronCores).

Computes: rowwise-fp8-quant(2:4-sparsify(relu(x)^2)) @ rowwise-fp8-quant(W).T -> bf16

Sharding: x rows split 4 ways (m-groups), W rows (= out cols) split 2 ways
(n-halves); core c handles m-group c % 4, n-half c // 4.

v2 restructure vs the original baseline:
  - Matmul channels (512-col n-slices) depend only on their own 4 W tiles,
    and PSUM accumulators live per (x-tile, channel), so the PE starts
    matmuls ~35us in instead of waiting for the whole W prep phase.
  - relu is folded away: the 2:4 threshold tree uses a relu-folded pair max
    (scalar_tensor_tensor (a max 0) max b), qs pairs stay raw (a negative
    qs can never exceed the nonneg pair-max side of the tree), and the
    value path gets its relu implicitly from the mask multiply (thr >= 0,
    and is_ge(x, 0) keeps only values that square to the right thing).
  - Engine rebalance: mask is_ge + row-max on GpSimd(Pool); tree, apply
    mult, scale smalls and dequant on DVE; quants on ACT.
  - W loads + W transposes ride the Activation HWDGE queue; x loads,
    x transposes, stores and small copies ride the SP queue.
  - W scales are kept in bf16 (~1e-3 extra rel err, well under the 2e-2
    gate) so the broadcast row lives in DRAM and the dequant operand is
    half-size.

Numerics notes (unchanged from baseline):
  - TRN fp8e4 max is +-240 (vs OCP e4m3fn +-448).  We quantize to +-224
    (scale' = 2*scale_ref); powers of two commute with RNE so the fp8
    rounding grid matches the reference exactly.
  - Transposes put K on partitions via bf16-bitcast fp8 pairs; layout:
    partition p, k-block b holds k = 256*b + 2*p + {0,1} adjacent bytes.
  - The matmul runs fp8 DoubleRowSwInterleave; stationary operand (x)
    expects column-reversed tiles, so the host pre-reverses x rows per
    128-tile; per-row x scales are un-reversed on chip (stream_shuffle +
    4 partition-block DMA copies).
"""
import sys
import os
import dataclasses

sys.path.insert(0, "/opt/trn_rl_repo")

import numpy as np
import ml_dtypes

import concourse.bass as bass
import concourse.mybir as mybir
from concourse.tile import TileContext
from concourse.bass_utils import run_bass_kernel_spmd

# ---------------------------------------------------------------------------
# Workaround: this environment's walrus rejects instructions with more than
# a couple of sync-wait conditions ("Too many sync wait commands").  Split
# excess waits onto NoOp instructions inserted before the offender.
import orjson as _orjson

_orig_to_json_bytes = bass.Bass.to_json_bytes
_LIMIT_DEFAULT = 1
_ws_counter = [0]


def _split_waits(doc):
    for fn in doc.get("functions", []):
        for blk in fn.get("blocks", []):
            insts = blk.get("instructions")
            if not insts:
                continue
            out = []
            changed = False
            for ins in insts:
                si = ins.get("sync_info")
                if si:
                    waits = si.get("on_wait") or []
                    if len(waits) > _LIMIT_DEFAULT:
                        excess = waits[:-_LIMIT_DEFAULT]
                        keep = waits[-_LIMIT_DEFAULT:]
                        for i in range(0, len(excess), _LIMIT_DEFAULT):
                            _ws_counter[0] += 1
                            out.append({
                                "name": f"I-waitsplit-{_ws_counter[0]}",
                                "engine": ins["engine"],
                                "opcode": "NoOp",
                                "ins": [],
                                "outs": [],
                                "sync_info": {
                                    "on_wait": excess[i:i + _LIMIT_DEFAULT],
                                    "on_update": [],
                                },
                            })
                        si["on_wait"] = keep
                        changed = True
                out.append(ins)
            if changed:
                blk["instructions"] = out
    return doc


def _patched_to_json_bytes(self):
    return _orjson.dumps(_split_waits(_orjson.loads(_orig_to_json_bytes(self))))


bass.Bass.to_json_bytes = _patched_to_json_bytes
# ---------------------------------------------------------------------------

F32 = mybir.dt.float32
FP8 = mybir.dt.float8e4
BF16 = mybir.dt.bfloat16
ALU = mybir.AluOpType
ACTF = mybir.ActivationFunctionType

M, K, NW = 8192, 4096, 4096
N_CORES = 8
MG, NH = 4, 2                  # m-groups x n-halves
MS, NS = M // MG, NW // NH     # 2048 x 2048 per-core output shard
NT = MS // 128                 # 16 x-tiles
WT = NS // 128                 # 16 w-tiles
KB = K // 256                  # 16 k-blocks of 256
NCH = 4                        # n channels of 512 cols each
SQRT224 = float(np.float32(np.sqrt(np.float32(224.0))))
INV224 = float(np.float32(1.0) / np.float32(224.0))

# config knobs (env for experiments)
W_QUEUE = os.environ.get("KV2_WQ", "act")        # 'act' | 'sp'
XQT_BUFS = int(os.environ.get("KV2_XQTB", "7"))
EARLY_T = XQT_BUFS - 1                            # tiles in the staggered ramp


def _pe_order():
    """Static matmul (tile, ch) order.  Channel c's W tiles are transposed
    by chain-iteration 4c+3; x tile t is ready by iteration t+1.  Tiles
    0..EARLY_T-1 run c0-c2 staggered by readiness, then their c3 wave
    (frees their xqT buffers), then the remaining tiles stream c0-c3."""
    p1 = [(t, c) for t in range(EARLY_T) for c in range(3)]
    p1.sort(key=lambda tc: (max(tc[0] + 1, 4 * tc[1] + 3), tc[1], tc[0]))
    p2 = [(t, 3) for t in range(EARLY_T)]
    p3 = [(t, c) for t in range(EARLY_T, NT) for c in range(NCH)]
    return p1 + p2 + p3


def _build_program():
    nc = bass.Bass()
    xs = nc.dram_tensor("xs", [MS, K], F32, kind="ExternalInput")
    ws = nc.dram_tensor("ws", [NS, K], F32, kind="ExternalInput")
    out = nc.dram_tensor("out", [MS, NS], BF16, kind="ExternalOutput")
    wsd = nc.dram_tensor("wsd", [1, NS], BF16, kind="Internal")

    rev32 = list(range(31, -1, -1))
    wdma = nc.scalar if W_QUEUE == "act" else nc.sync

    with TileContext(nc) as tc:
        with tc.tile_pool(name="persist", bufs=1) as cpool, \
             tc.tile_pool(name="work", bufs=1) as pool, \
             tc.tile_pool(name="psum", bufs=8, space="PSUM") as psp:

            WqT = cpool.tile([128, KB, NS], BF16)     # 8 MB
            WscaleB = cpool.tile([128, NS], BF16)     # 512 KB
            xnats = cpool.tile([128, NT], F32)        # un-reversed x scales

            # ---------------- W pipeline ----------------
            wtiles = {}
            wscales = {}

            def w_load(wt):
                wtile = pool.tile([128, K], F32, tag="wstage", bufs=2,
                                  name=f"wtile_{wt}")
                nc.sync.dma_start(out=wtile[:], in_=ws[wt * 128:(wt + 1) * 128])
                wtiles[wt] = wtile

            def w_absmax(wt):
                wtile = wtiles[wt]
                wabs = pool.tile([128, 1], F32, tag="sA", bufs=2, name=f"wabs_{wt}")
                nc.vector.tensor_reduce(out=wabs[:], in_=wtile[:],
                                        axis=mybir.AxisListType.X, op=ALU.max,
                                        apply_absolute_value=True)
                winv = pool.tile([128, 1], F32, tag="sB", bufs=2, name=f"winv_{wt}")
                nc.vector.reciprocal(out=winv[:], in_=wabs[:])
                winv2 = pool.tile([128, 1], F32, tag="sC", bufs=2, name=f"winv2_{wt}")
                nc.vector.tensor_scalar_mul(out=winv2[:], in0=winv[:], scalar1=224.0)
                wscale = pool.tile([128, 1], BF16, tag="sD", bufs=2,
                                   name=f"wscale_{wt}")
                nc.vector.tensor_scalar_mul(out=wscale[:], in0=wabs[:],
                                            scalar1=INV224)
                wscales[wt] = (winv2, wscale)

            def w_srow(wt):
                _, wscale = wscales[wt]
                nc.scalar.dma_start(out=wsd[0:1, wt * 128:(wt + 1) * 128],
                                  in_=wscale[:])

            def w_quant(wt):
                wtile = wtiles.pop(wt)
                winv2, _ = wscales[wt]
                wq = pool.tile([128, K], FP8, tag="wq8", bufs=1, name=f"wq_{wt}")
                nc.scalar.activation(out=wq[:], in_=wtile[:], func=ACTF.Copy,
                                     scale=winv2[:])
                wtiles[wt] = wq   # now holds the quantized tile

            def w_xpose(wt):
                wq = wtiles.pop(wt)
                wdma.dma_start_transpose(WqT[:, :, wt * 128:(wt + 1) * 128],
                                         wq[:].bitcast(BF16))

            def bcast(c):
                cs, ce = c * 512, (c + 1) * 512
                src = wsd[0:1, cs:ce]
                src = dataclasses.replace(src, ap=[[0, 128]] + list(src.ap[1:]))
                nc.scalar.dma_start(out=WscaleB[:, cs:ce], in_=src)

            # ---------------- X pipeline ----------------
            xtiles = {}
            xthrs = {}
            xsqs = {}
            xqs = {}
            xqts = {}

            def x_load(mt):
                xt = pool.tile([128, K], F32, tag="xstage", bufs=2, name=f"xt_{mt}")
                nc.sync.dma_start(out=xt[:], in_=xs[mt * 128:(mt + 1) * 128])
                xtiles[mt] = xt

            def x_tree(mt):
                xt = xtiles[mt]
                x2 = xt[:].rearrange("p (g two) -> p g two", two=2)
                # relu folded into the pair max: pr = max(max(a,0), b) [DVE]
                pr = pool.tile([128, K // 2], F32, tag="pr", bufs=1, name=f"pr_{mt}")
                nc.vector.scalar_tensor_tensor(out=pr[:], in0=x2[:, :, 0],
                                               scalar=0.0, in1=x2[:, :, 1],
                                               op0=ALU.max, op1=ALU.max)
                # raw pair min path on Pool (negatives lose to u1 >= 0 anyway)
                qs = pool.tile([128, K // 2], F32, tag="qs", bufs=1, name=f"qs_{mt}")
                nc.vector.tensor_tensor(out=qs[:], in0=x2[:, :, 0], in1=x2[:, :, 1],
                                        op=ALU.min)
                pr2 = pr[:].rearrange("p (g two) -> p g two", two=2)
                qs2 = qs[:].rearrange("p (g two) -> p g two", two=2)
                u1 = pool.tile([128, K // 4], F32, tag="u1", bufs=1, name=f"u1_{mt}")
                nc.vector.tensor_tensor(out=u1[:], in0=pr2[:, :, 0], in1=pr2[:, :, 1],
                                        op=ALU.min)
                thr = pool.tile([128, K // 4], F32, tag="thr", bufs=2, name=f"thr_{mt}")
                nc.vector.tensor_tensor(out=thr[:], in0=qs2[:, :, 0], in1=qs2[:, :, 1],
                                        op=ALU.max)
                nc.vector.tensor_tensor(out=thr[:], in0=thr[:], in1=u1[:],
                                        op=ALU.max)
                xthrs[mt] = (pr, thr)

            def x_rmax(mt):
                pr, _ = xthrs[mt]
                rmax = pool.tile([128, 1], F32, tag="sE", bufs=2, name=f"rmax_{mt}")
                nc.vector.tensor_reduce(out=rmax[:], in_=pr[:],
                                        axis=mybir.AxisListType.X, op=ALU.max)
                return rmax

            def x_scales(mt, rmax):
                rm2 = pool.tile([128, 1], F32, tag="sF", bufs=2, name=f"rm2_{mt}")
                nc.vector.tensor_scalar_max(out=rm2[:], in0=rmax[:], scalar1=1e-5)
                rrec = pool.tile([128, 1], F32, tag="sG", bufs=2, name=f"rrec_{mt}")
                nc.vector.reciprocal(out=rrec[:], in_=rm2[:])
                sq = pool.tile([128, 1], F32, tag="sH", bufs=2, name=f"sq_{mt}")
                nc.vector.tensor_scalar_mul(out=sq[:], in0=rrec[:], scalar1=SQRT224)
                xsc = pool.tile([128, 1], F32, tag="sI", bufs=2, name=f"xsc_{mt}")
                nc.vector.tensor_tensor(out=xsc[:], in0=rmax[:], in1=rmax[:],
                                        op=ALU.mult)
                xsc2 = pool.tile([128, 1], F32, tag="sJ", bufs=2, name=f"xsc2_{mt}")
                nc.vector.tensor_scalar_mul(out=xsc2[:], in0=xsc[:], scalar1=INV224)
                xsh = pool.tile([128, 1], F32, tag="sK", bufs=2, name=f"xsh_{mt}")
                nc.vector.stream_shuffle(out=xsh[:], in_=xsc2[:], mask=rev32)
                xsqs[mt] = sq
                return xsh

            def x_nat(mt, xsh):
                # un-reverse across the four 32-partition blocks
                for q in range(4):
                    nc.scalar.dma_start(out=xnats[32 * (3 - q):32 * (4 - q), mt:mt + 1],
                                      in_=xsh[32 * q:32 * (q + 1)])

            def x_finish(mt):
                xt = xtiles.pop(mt)
                _, thr = xthrs.pop(mt)
                mask = pool.tile([128, K], FP8, tag="mask", bufs=1, name=f"mask_{mt}")
                x4 = xt[:].rearrange("p (g four) -> p g four", four=4)
                m4 = mask[:].rearrange("p (g four) -> p g four", four=4)
                tb = thr[:].rearrange("p (g one) -> p g one", one=1)
                tb = dataclasses.replace(tb, ap=[tb.ap[0], tb.ap[1], [0, 4]])
                nc.vector.tensor_tensor(out=m4[:], in0=x4[:], in1=tb, op=ALU.is_ge)
                nc.vector.tensor_tensor(out=xt[:], in0=xt[:], in1=mask[:],
                                        op=ALU.mult)
                sq = xsqs.pop(mt)
                xq = pool.tile([128, K], FP8, tag="xq8", bufs=1, name=f"xq_{mt}")
                nc.scalar.activation(out=xq[:], in_=xt[:], func=ACTF.Square,
                                     scale=sq[:])
                xqs[mt] = xq

            def x_xpose(mt):
                xq = xqs.pop(mt)
                xqT = pool.tile([128, KB, 128], BF16, tag="xqT", bufs=XQT_BUFS,
                                name=f"xqT_{mt}")
                nc.scalar.dma_start_transpose(xqT[:], xq[:].bitcast(BF16))
                xqts[mt] = xqT

            # ---------------- matmul / dequant / store ----------------
            accs = {}

            def mm(mt, ch):
                if mt not in xqts:
                    x_xpose(mt)
                xqT = xqts[mt]
                acc = psp.tile([128, 512], F32, tag="acc", name=f"acc_{mt}_{ch}")
                wq8 = WqT[:].bitcast(FP8)   # [128, KB, 2*NS]
                xq8 = xqT[:].bitcast(FP8)   # [128, KB, 256]
                for blk in range(KB):
                    lhs = xq8[:, blk, :]
                    rhs = wq8[:, blk, ch * 1024:(ch + 1) * 1024].rearrange(
                        "p (n two) -> p two n", two=2)
                    nc.tensor.matmul(acc[:], lhs, rhs,
                                     start=(blk == 0), stop=(blk == KB - 1),
                                     perf_mode=mybir.MatmulPerfMode.DoubleRowSwInterleave)
                accs[(mt, ch)] = acc
                if ch == NCH - 1:
                    xqts.pop(mt)  # last reader emitted; frees the xqT slot

            def dq_store(mt, ch):
                acc = accs.pop((mt, ch))
                # acc * xnat on ACT (per-partition scale), then a packed-bf16
                # 2x multiply by the broadcast W scales on DVE
                dqt = pool.tile([128, 512], BF16, tag="dqt", bufs=2,
                                name=f"dqt_{mt}_{ch}")
                nc.scalar.activation(out=dqt[:], in_=acc[:], func=ACTF.Copy,
                                     scale=xnats[:, mt:mt + 1])
                ost = pool.tile([128, 512], BF16, tag="ost", bufs=2,
                                name=f"ost_{mt}_{ch}")
                nc.vector.tensor_tensor(
                    out=ost[:], in0=dqt[:],
                    in1=WscaleB[:, ch * 512:(ch + 1) * 512], op=ALU.mult)
                nc.scalar.dma_start(
                    out=out[mt * 128:(mt + 1) * 128, ch * 512:(ch + 1) * 512],
                    in_=ost[:])

            # ---------------- emission schedule ----------------
            pe_list = _pe_order()
            pe_pos = 0
            pending_dq = []
            x_ready = set()
            ch_ready = set()

            def pump_pe(budget):
                """Emit up to `budget` matmul (t,c) pairs (in pe_list order,
                gated on emitted deps); dequants trail by one pump call so
                the DVE never head-of-line blocks on an unfinished matmul."""
                nonlocal pe_pos
                while pending_dq:
                    dq_store(*pending_dq.pop(0))
                emitted = 0
                while pe_pos < len(pe_list) and emitted < budget:
                    t, c = pe_list[pe_pos]
                    if t not in x_ready or c not in ch_ready:
                        break
                    mm(t, c)
                    pending_dq.append((t, c))
                    pe_pos += 1
                    emitted += 1

            # prologue
            w_load(0)
            x_load(0)
            w_load(1)

            for i in range(NT + 3):
                # W chain first: its DVE work has no intra-iteration deps,
                # and the sooner wq tiles transpose, the sooner PE channels
                # unlock.
                if i < WT:
                    w_absmax(i)
                    w_srow(i)
                    w_quant(i)
                    w_xpose(i)
                    if i % 4 == 3:
                        c = i // 4
                        bcast(c)
                        ch_ready.add(c)
                # X chain: finish tile i-1 (its thr/mask deps are a full
                # iteration old, so the DVE never stalls).
                if 1 <= i <= NT:
                    t = i - 1
                    x_finish(t)
                    x_ready.add(t)
                # loads last: the quants that free their staging slots are
                # already emitted, so the pure-load SP queue never waits on
                # anything further than one iteration out.
                if i + 2 < WT:
                    w_load(i + 2)
                if i + 1 < NT:
                    x_load(i + 1)
                if i < NT:
                    x_tree(i)
                    rmax = x_rmax(i)
                    xsh = x_scales(i, rmax)
                    x_nat(i, xsh)
                pump_pe(3)
            # drain remaining matmuls + dequants
            while pe_pos < len(pe_list) or pending_dq:
                prev = pe_pos
                pump_pe(4)
                if pe_pos == prev and pe_pos < len(pe_list):
                    raise RuntimeError(
                        f"pe schedule stalled at {pe_pos}: {pe_list[pe_pos]}")

    return nc




# ===========================================================================
# v3: cross-core dedup of the x/W prep via AllGather collectives.
#
# Each x row-block was sparsified+quantized on BOTH n-half cores, and each
# W row-block quantized on all FOUR m-group cores.  v3 assigns each core a
# disjoint slice of the prep work and exchanges the quantized+transposed
# fp8 tiles through DRAM AllGathers:
#   - x: core (g,h) preps m-tiles {h*8+p} of its m-group; pairs {c, c+4}
#     gather per-tile bundles (xqT bytes + the un-reversed row scale).
#     Gathered slot 0 = global tile p, slot 1 = global tile 8+p on BOTH
#     cores, so all addressing stays SPMD-static.
#   - W: core (g,h) preps global W tiles {4j+g} (stride-4 interleave), so
#     the j-th gather over the quad {4h.. } delivers exactly channel j
#     (n columns [j*512,(j+1)*512)), keeping the per-channel PE ramp.
# DMA trigger pressure: loads on SP; transposes+quants on ACT; bounces,
# reloads, stores and small copies on the GpSimd software DGE (idle
# engine).  Stores are batched per (tile, ch-pair) rows.
# ===========================================================================

XP = 8        # x tiles prepped per core
WP = 4        # w tiles prepped per core
XGROUPS = [[0, 4], [1, 5], [2, 6], [3, 7]]
WGROUPS = [[0, 1, 2, 3], [4, 5, 6, 7]]


def _pe_order_v3():
    order = []
    for p in range(XP):
        for c in range(NCH):
            order.append((p, c))
        for c in range(NCH):
            order.append((8 + p, c))
    return order


def _build_program_v3():
    nc = bass.Bass(num_devices=N_CORES)
    xs = nc.dram_tensor("xs", [XP * 128, K], F32, kind="ExternalInput")
    ws = nc.dram_tensor("ws", [WP * 128, K], F32, kind="ExternalInput")
    out = nc.dram_tensor("out", [MS, NS], BF16, kind="ExternalOutput")
    wsd = nc.dram_tensor("wsd", [1, WP * 128], BF16, kind="Internal")
    wsg = nc.dram_tensor("wsg", [4, 1, WP * 128], BF16, kind="Internal")
    wqbo = [nc.dram_tensor(f"wqbo{j}", [128, KB * 128], BF16, kind="Internal")
            for j in range(WP)]
    wqbi = [nc.dram_tensor(f"wqbi{j}", [4, 128, KB * 128], BF16, kind="Internal")
            for j in range(WP)]
    XBN = KB * 128 + 2   # xqT bytes (bf16 cols) + scale
    xqbo = [nc.dram_tensor(f"xqbo{p}", [128, XBN], BF16, kind="Internal")
            for p in range(XP)]
    xqbi = [nc.dram_tensor(f"xqbi{p}", [2, 128, XBN], BF16, kind="Internal")
            for p in range(XP)]

    rev32 = list(range(31, -1, -1))

    with TileContext(nc) as tc:
        with tc.tile_pool(name="persist", bufs=1) as cpool, \
             tc.tile_pool(name="work", bufs=1) as pool, \
             tc.tile_pool(name="psum", bufs=8, space="PSUM") as psp:

            WqT = cpool.tile([128, KB, NS], BF16)     # 8 MB
            WscaleB = cpool.tile([128, NS], BF16)     # 512 KB
            xnats = cpool.tile([128, NT], F32)        # global-tile row scales

            # ---------------- W pipeline (front-loaded, 4 own tiles) -------
            wtiles = {}
            wscales = {}

            def w_load(j):
                wtile = pool.tile([128, K], F32, tag="stage", bufs=4,
                                  name=f"wtile_{j}")
                nc.sync.dma_start(out=wtile[:], in_=ws[j * 128:(j + 1) * 128])
                wtiles[j] = wtile

            def w_prep(j):
                wtile = wtiles[j]
                wabs = pool.tile([128, 1], F32, tag="sA", bufs=2, name=f"wabs_{j}")
                nc.vector.tensor_reduce(out=wabs[:], in_=wtile[:],
                                        axis=mybir.AxisListType.X, op=ALU.max,
                                        apply_absolute_value=True)
                winv = pool.tile([128, 1], F32, tag="sB", bufs=2, name=f"winv_{j}")
                nc.vector.reciprocal(out=winv[:], in_=wabs[:])
                winv2 = pool.tile([128, 1], F32, tag="sC", bufs=2, name=f"winv2_{j}")
                nc.vector.tensor_scalar_mul(out=winv2[:], in0=winv[:], scalar1=224.0)
                wscale = pool.tile([128, 1], BF16, tag="sD", bufs=2,
                                   name=f"wscale_{j}")
                nc.vector.tensor_scalar_mul(out=wscale[:], in0=wabs[:],
                                            scalar1=INV224)
                nc.gpsimd.dma_start(out=wsd[0:1, j * 128:(j + 1) * 128],
                                    in_=wscale[:])
                wtile2 = wtiles.pop(j)
                wq = pool.tile([128, K], FP8, tag="wq8", bufs=1, name=f"wq_{j}")
                nc.scalar.activation(out=wq[:], in_=wtile2[:], func=ACTF.Copy,
                                     scale=winv2[:])
                wqt = pool.tile([128, KB, 128], BF16, tag="qto", bufs=2,
                                name=f"wqt_{j}")
                nc.scalar.dma_start_transpose(wqt[:], wq[:].bitcast(BF16))
                nc.gpsimd.dma_start(out=wqbo[j][:], in_=wqt[:])
                nc.gpsimd.collective_compute(
                    "AllGather", ALU.bypass, replica_groups=WGROUPS,
                    ins=[wqbo[j][:].opt()], outs=[wqbi[j][:].opt()])

            def w_reload(j):
                for g2 in range(4):
                    wt = 4 * j + g2
                    nc.gpsimd.dma_start(
                        out=WqT[:, :, wt * 128:(wt + 1) * 128],
                        in_=wqbi[j][g2].rearrange("p (kb n) -> p kb n", kb=KB))

            def w_scales_cc():
                nc.gpsimd.collective_compute(
                    "AllGather", ALU.bypass, replica_groups=WGROUPS,
                    ins=[wsd[:].opt()], outs=[wsg[:].opt()])

            def w_scales_bcast():
                for j in range(WP):
                    for g2 in range(4):
                        wt = 4 * j + g2
                        src = wsg[g2, 0:1, j * 128:(j + 1) * 128]
                        src = dataclasses.replace(
                            src, ap=[[0, 128]] + list(src.ap[1:]))
                        nc.gpsimd.dma_start(
                            out=WscaleB[:, wt * 128:(wt + 1) * 128], in_=src)

            # ---------------- X pipeline (8 own tiles + pair gathers) ------
            xtiles = {}
            xthrs = {}
            xsqs = {}
            xqts = {}

            def x_load(p):
                xt = pool.tile([128, K], F32, tag="stage", bufs=4, name=f"xt_{p}")
                nc.sync.dma_start(out=xt[:], in_=xs[p * 128:(p + 1) * 128])
                xtiles[p] = xt

            def x_tree(p):
                xt = xtiles[p]
                x2 = xt[:].rearrange("p (g two) -> p g two", two=2)
                pr = pool.tile([128, K // 2], F32, tag="pr", bufs=1, name=f"pr_{p}")
                nc.vector.scalar_tensor_tensor(out=pr[:], in0=x2[:, :, 0],
                                               scalar=0.0, in1=x2[:, :, 1],
                                               op0=ALU.max, op1=ALU.max)
                qs = pool.tile([128, K // 2], F32, tag="qs", bufs=1, name=f"qs_{p}")
                nc.vector.tensor_tensor(out=qs[:], in0=x2[:, :, 0], in1=x2[:, :, 1],
                                        op=ALU.min)
                rmax = pool.tile([128, 1], F32, tag="sE", bufs=2, name=f"rmax_{p}")
                nc.vector.tensor_reduce(out=rmax[:], in_=pr[:],
                                        axis=mybir.AxisListType.X, op=ALU.max)
                pr2 = pr[:].rearrange("p (g two) -> p g two", two=2)
                qs2 = qs[:].rearrange("p (g two) -> p g two", two=2)
                u1 = pr[:, 0:K // 4]
                nc.vector.tensor_tensor(out=u1, in0=pr2[:, :, 0], in1=pr2[:, :, 1],
                                        op=ALU.min)
                thr = qs[:, 0:K // 4]
                nc.vector.tensor_tensor(out=thr, in0=qs2[:, :, 0], in1=qs2[:, :, 1],
                                        op=ALU.max)
                nc.vector.tensor_tensor(out=thr, in0=thr, in1=u1,
                                        op=ALU.max)
                xthrs[p] = (pr, qs)
                rm2 = pool.tile([128, 1], F32, tag="sF", bufs=2, name=f"rm2_{p}")
                nc.vector.tensor_scalar_max(out=rm2[:], in0=rmax[:], scalar1=1e-5)
                rrec = pool.tile([128, 1], F32, tag="sG", bufs=2, name=f"rrec_{p}")
                nc.vector.reciprocal(out=rrec[:], in_=rm2[:])
                sq = pool.tile([128, 1], F32, tag="sH", bufs=2, name=f"sq_{p}")
                nc.vector.tensor_scalar_mul(out=sq[:], in0=rrec[:], scalar1=SQRT224)
                xsc = pool.tile([128, 1], F32, tag="sI", bufs=2, name=f"xsc_{p}")
                nc.vector.tensor_tensor(out=xsc[:], in0=rmax[:], in1=rmax[:],
                                        op=ALU.mult)
                xsc2 = pool.tile([128, 1], F32, tag="sJ", bufs=2, name=f"xsc2_{p}")
                nc.vector.tensor_scalar_mul(out=xsc2[:], in0=xsc[:], scalar1=INV224)
                xsh = pool.tile([128, 1], F32, tag="sK", bufs=2, name=f"xsh_{p}")
                nc.vector.stream_shuffle(out=xsh[:], in_=xsc2[:], mask=rev32)
                xsqs[p] = (sq, xsh)

            def x_finish(p):
                xt = xtiles.pop(p)
                _, qs = xthrs.pop(p)
                thr = qs[:, 0:K // 4]
                mask = pool.tile([128, K], FP8, tag="mask", bufs=1, name=f"mask_{p}")
                x4 = xt[:].rearrange("p (g four) -> p g four", four=4)
                m4 = mask[:].rearrange("p (g four) -> p g four", four=4)
                tb = thr.rearrange("p (g one) -> p g one", one=1)
                tb = dataclasses.replace(tb, ap=[tb.ap[0], tb.ap[1], [0, 4]])
                nc.vector.tensor_tensor(out=m4[:], in0=x4[:], in1=tb, op=ALU.is_ge)
                nc.vector.tensor_tensor(out=xt[:], in0=xt[:], in1=mask[:],
                                        op=ALU.mult)
                sq, xsh = xsqs.pop(p)
                xq = pool.tile([128, K], FP8, tag="xq8", bufs=1, name=f"xq_{p}")
                nc.scalar.activation(out=xq[:], in_=xt[:], func=ACTF.Square,
                                     scale=sq[:])
                xqto = pool.tile([128, KB, 128], BF16, tag="qto", bufs=2,
                                 name=f"xqto_{p}")
                nc.scalar.dma_start_transpose(xqto[:], xq[:].bitcast(BF16))
                # un-reverse the row scale into a compact tile, then bounce
                # bundle = [xqT bytes | scale-as-2-bf16 | pad]
                xno = pool.tile([128, 1], F32, tag="sL", bufs=2, name=f"xno_{p}")
                for q in range(4):
                    nc.gpsimd.dma_start(out=xno[32 * (3 - q):32 * (4 - q)],
                                        in_=xsh[32 * q:32 * (q + 1)])
                nc.gpsimd.dma_start(out=xqbo[p][:, 0:KB * 128], in_=xqto[:])
                nc.gpsimd.dma_start(out=xqbo[p][:, KB * 128:KB * 128 + 2],
                                    in_=xno[:].bitcast(BF16))
                nc.gpsimd.collective_compute(
                    "AllGather", ALU.bypass, replica_groups=XGROUPS,
                    ins=[xqbo[p][:].opt()], outs=[xqbi[p][:].opt()])

            def x_reload(p):
                for i in range(2):
                    t = 8 * i + p
                    xqT = pool.tile([128, KB, 128], BF16, tag="xqT", bufs=6,
                                    name=f"xqT_{t}")
                    nc.gpsimd.dma_start(
                        out=xqT[:],
                        in_=xqbi[p][i, :, 0:KB * 128].rearrange(
                            "p (kb n) -> p kb n", kb=KB))
                    xqts[t] = xqT
                    nc.gpsimd.dma_start(
                        out=xnats[:, t:t + 1],
                        in_=xqbi[p][i, :, KB * 128:KB * 128 + 2].bitcast(F32))

            # ---------------- matmul / dequant / store ---------------------
            accs = {}
            osts = {}

            def mm(t, ch):
                xqT = xqts[t]
                acc = psp.tile([128, 512], F32, tag="acc", name=f"acc_{t}_{ch}")
                wq8 = WqT[:].bitcast(FP8)
                xq8 = xqT[:].bitcast(FP8)
                for blk in range(KB):
                    lhs = xq8[:, blk, :]
                    rhs = wq8[:, blk, ch * 1024:(ch + 1) * 1024].rearrange(
                        "p (n two) -> p two n", two=2)
                    nc.tensor.matmul(acc[:], lhs, rhs,
                                     start=(blk == 0), stop=(blk == KB - 1),
                                     perf_mode=mybir.MatmulPerfMode.DoubleRowSwInterleave)
                accs[(t, ch)] = acc
                if ch == NCH - 1:
                    xqts.pop(t)

            def dq(t, ch):
                acc = accs.pop((t, ch))
                dqt = pool.tile([128, 512], BF16, tag="dqt", bufs=1,
                                name=f"dqt_{t}_{ch}")
                nc.scalar.activation(out=dqt[:], in_=acc[:], func=ACTF.Copy,
                                     scale=xnats[:, t:t + 1])
                if t not in osts:
                    osts[t] = pool.tile([128, NS], BF16, tag="ost", bufs=2,
                                        name=f"ost_{t}")
                ost = osts[t]
                nc.vector.tensor_tensor(
                    out=ost[:, ch * 512:(ch + 1) * 512], in0=dqt[:],
                    in1=WscaleB[:, ch * 512:(ch + 1) * 512], op=ALU.mult)
                if ch == NCH - 1:
                    nc.gpsimd.dma_start(out=out[t * 128:(t + 1) * 128], in_=ost[:])
                    osts.pop(t)

            # ---------------- emission -------------------------------------
            pe_list = _pe_order_v3()
            pe_pos = 0
            pending_dq = []
            x_ready = set()
            ch_ready = set()

            def pump_pe(budget):
                nonlocal pe_pos
                while pending_dq:
                    dq(*pending_dq.pop(0))
                emitted = 0
                while pe_pos < len(pe_list) and emitted < budget:
                    t, c = pe_list[pe_pos]
                    if t not in x_ready or c not in ch_ready:
                        break
                    mm(t, c)
                    pending_dq.append((t, c))
                    pe_pos += 1
                    emitted += 1

            # W phase: all four own tiles, fully pipelined through the
            # gathers; x loads stream behind the W loads on SP.
            w_load(0)
            w_load(1)
            x_load(0)
            for j in range(WP):
                w_prep(j)
                if j >= 1:
                    w_reload(j - 1)
                    ch_ready.add(j - 1)
                if j + 2 < WP:
                    w_load(j + 2)
                else:
                    x_load(1 + (j + 2 - WP))
            w_scales_cc()
            w_reload(WP - 1)
            ch_ready.add(WP - 1)
            w_scales_bcast()

            for i in range(XP + 3):
                if 1 <= i <= XP:
                    p = i - 1
                    x_finish(p)
                if 2 <= i <= XP + 1:
                    p = i - 2
                    x_reload(p)
                    x_ready.add(p)
                    x_ready.add(8 + p)
                if i + 3 < XP:
                    x_load(i + 3)
                if i < XP:
                    x_tree(i)
                pump_pe(4)
            while pe_pos < len(pe_list) or pending_dq:
                prev = pe_pos
                pump_pe(6)
                if pe_pos == prev and pe_pos < len(pe_list):
                    raise RuntimeError(
                        f"pe schedule stalled at {pe_pos}: {pe_list[pe_pos]}")

    return nc


def _run_v3(x, W, trace=False):
    x = np.ascontiguousarray(x, dtype=np.float32)
    W = np.ascontiguousarray(W, dtype=np.float32)
    nc = _get_nc()
    in_maps = []
    for c in range(N_CORES):
        g, h = c % MG, c // MG
        xg = x[g * MS:(g + 1) * MS].reshape(NT, 128, K)[:, ::-1, :]
        xsh = xg[h * XP:(h + 1) * XP].reshape(XP * 128, K)
        wrows = np.concatenate(
            [W[h * NS + (4 * j + g) * 128: h * NS + (4 * j + g + 1) * 128]
             for j in range(WP)], axis=0)
        in_maps.append({
            "xs": np.ascontiguousarray(xsh),
            "ws": np.ascontiguousarray(wrows),
        })
    res = run_bass_kernel_spmd(nc, in_maps, core_ids=list(range(N_CORES)),
                               trace=trace)
    outf = np.empty((M, NW), dtype=ml_dtypes.bfloat16)
    for c in range(N_CORES):
        g, h = c % MG, c // MG
        outf[g * MS:(g + 1) * MS, h * NS:(h + 1) * NS] = res.results[c]["out"]
    return outf, res

_cached_nc = None
KV3 = os.environ.get("KV3", "1") == "1"


def _get_nc():
    global _cached_nc
    if _cached_nc is None:
        _cached_nc = _build_program_v3() if KV3 else _build_program()
    return _cached_nc


def _run(x, W, trace=False):
    if KV3:
        return _run_v3(x, W, trace=trace)
    x = np.ascontiguousarray(x, dtype=np.float32)
    W = np.ascontiguousarray(W, dtype=np.float32)
    assert x.shape == (M, K) and W.shape == (NW, K)
    nc = _get_nc()
    in_maps = []
    for c in range(N_CORES):
        g, h = c % MG, c // MG
        xsh = x[g * MS:(g + 1) * MS].reshape(NT, 128, K)[:, ::-1, :].reshape(MS, K)
        in_maps.append({
            "xs": np.ascontiguousarray(xsh),
            "ws": W[h * NS:(h + 1) * NS],
        })
    res = run_bass_kernel_spmd(nc, in_maps, core_ids=list(range(N_CORES)),
                               trace=trace)
    outf = np.empty((M, NW), dtype=ml_dtypes.bfloat16)
    for c in range(N_CORES):
        g, h = c % MG, c // MG
        outf[g * MS:(g + 1) * MS, h * NS:(h + 1) * NS] = res.results[c]["out"]
    return outf, res


def kernel(x, W):
    out, _ = _run(x, W, trace=False)
    return out


# revision 16
# speedup vs baseline: 1.0861x; 1.0861x over previous
"""FP8 semi-sparse activation linear kernel for Trainium2 (8 NeuronCores).

Computes: rowwise-fp8-quant(2:4-sparsify(relu(x)^2)) @ rowwise-fp8-quant(W).T -> bf16

Sharding: x rows split 4 ways (m-groups), W rows (= out cols) split 2 ways
(n-halves); core c handles m-group c % 4, n-half c // 4.

v2 restructure vs the original baseline:
  - Matmul channels (512-col n-slices) depend only on their own 4 W tiles,
    and PSUM accumulators live per (x-tile, channel), so the PE starts
    matmuls ~35us in instead of waiting for the whole W prep phase.
  - relu is folded away: the 2:4 threshold tree uses a relu-folded pair max
    (scalar_tensor_tensor (a max 0) max b), qs pairs stay raw (a negative
    qs can never exceed the nonneg pair-max side of the tree), and the
    value path gets its relu implicitly from the mask multiply (thr >= 0,
    and is_ge(x, 0) keeps only values that square to the right thing).
  - Engine rebalance: mask is_ge + row-max on GpSimd(Pool); tree, apply
    mult, scale smalls and dequant on DVE; quants on ACT.
  - W loads + W transposes ride the Activation HWDGE queue; x loads,
    x transposes, stores and small copies ride the SP queue.
  - W scales are kept in bf16 (~1e-3 extra rel err, well under the 2e-2
    gate) so the broadcast row lives in DRAM and the dequant operand is
    half-size.

Numerics notes (unchanged from baseline):
  - TRN fp8e4 max is +-240 (vs OCP e4m3fn +-448).  We quantize to +-224
    (scale' = 2*scale_ref); powers of two commute with RNE so the fp8
    rounding grid matches the reference exactly.
  - Transposes put K on partitions via bf16-bitcast fp8 pairs; layout:
    partition p, k-block b holds k = 256*b + 2*p + {0,1} adjacent bytes.
  - The matmul runs fp8 DoubleRowSwInterleave; stationary operand (x)
    expects column-reversed tiles, so the host pre-reverses x rows per
    128-tile; per-row x scales are un-reversed on chip (stream_shuffle +
    4 partition-block DMA copies).
"""
import sys
import os
import dataclasses

sys.path.insert(0, "/opt/trn_rl_repo")

import numpy as np
import ml_dtypes

import concourse.bass as bass
import concourse.mybir as mybir
from concourse.tile import TileContext
from concourse.bass_utils import run_bass_kernel_spmd

# ---------------------------------------------------------------------------
# Workaround: this environment's walrus rejects instructions with more than
# a couple of sync-wait conditions ("Too many sync wait commands").  Split
# excess waits onto NoOp instructions inserted before the offender.
import orjson as _orjson

_orig_to_json_bytes = bass.Bass.to_json_bytes
_LIMIT_DEFAULT = 1
_ws_counter = [0]


def _split_waits(doc):
    for fn in doc.get("functions", []):
        for blk in fn.get("blocks", []):
            insts = blk.get("instructions")
            if not insts:
                continue
            out = []
            changed = False
            for ins in insts:
                si = ins.get("sync_info")
                if si:
                    waits = si.get("on_wait") or []
                    if len(waits) > _LIMIT_DEFAULT:
                        excess = waits[:-_LIMIT_DEFAULT]
                        keep = waits[-_LIMIT_DEFAULT:]
                        for i in range(0, len(excess), _LIMIT_DEFAULT):
                            _ws_counter[0] += 1
                            out.append({
                                "name": f"I-waitsplit-{_ws_counter[0]}",
                                "engine": ins["engine"],
                                "opcode": "NoOp",
                                "ins": [],
                                "outs": [],
                                "sync_info": {
                                    "on_wait": excess[i:i + _LIMIT_DEFAULT],
                                    "on_update": [],
                                },
                            })
                        si["on_wait"] = keep
                        changed = True
                out.append(ins)
            if changed:
                blk["instructions"] = out
    return doc


def _patched_to_json_bytes(self):
    return _orjson.dumps(_split_waits(_orjson.loads(_orig_to_json_bytes(self))))


bass.Bass.to_json_bytes = _patched_to_json_bytes
# ---------------------------------------------------------------------------

F32 = mybir.dt.float32
FP8 = mybir.dt.float8e4
BF16 = mybir.dt.bfloat16
ALU = mybir.AluOpType
ACTF = mybir.ActivationFunctionType

M, K, NW = 8192, 4096, 4096
N_CORES = 8
MG, NH = 4, 2                  # m-groups x n-halves
MS, NS = M // MG, NW // NH     # 2048 x 2048 per-core output shard
NT = MS // 128                 # 16 x-tiles
WT = NS // 128                 # 16 w-tiles
KB = K // 256                  # 16 k-blocks of 256
NCH = 4                        # n channels of 512 cols each
SQRT224 = float(np.float32(np.sqrt(np.float32(224.0))))
INV224 = float(np.float32(1.0) / np.float32(224.0))

# config knobs (env for experiments)
W_QUEUE = os.environ.get("KV2_WQ", "act")        # 'act' | 'sp'
XQT_BUFS = int(os.environ.get("KV2_XQTB", "7"))
EARLY_T = XQT_BUFS - 1                            # tiles in the staggered ramp


def _pe_order():
    """Static matmul (tile, ch) order.  Channel c's W tiles are transposed
    by chain-iteration 4c+3; x tile t is ready by iteration t+1.  Tiles
    0..EARLY_T-1 run c0-c2 staggered by readiness, then their c3 wave
    (frees their xqT buffers), then the remaining tiles stream c0-c3."""
    p1 = [(t, c) for t in range(EARLY_T) for c in range(3)]
    p1.sort(key=lambda tc: (max(tc[0] + 1, 4 * tc[1] + 3), tc[1], tc[0]))
    p2 = [(t, 3) for t in range(EARLY_T)]
    p3 = [(t, c) for t in range(EARLY_T, NT) for c in range(NCH)]
    return p1 + p2 + p3


def _build_program():
    nc = bass.Bass()
    xs = nc.dram_tensor("xs", [MS, K], F32, kind="ExternalInput")
    ws = nc.dram_tensor("ws", [NS, K], F32, kind="ExternalInput")
    out = nc.dram_tensor("out", [MS, NS], BF16, kind="ExternalOutput")
    wsd = nc.dram_tensor("wsd", [1, NS], BF16, kind="Internal")

    rev32 = list(range(31, -1, -1))
    wdma = nc.scalar if W_QUEUE == "act" else nc.sync

    with TileContext(nc) as tc:
        with tc.tile_pool(name="persist", bufs=1) as cpool, \
             tc.tile_pool(name="work", bufs=1) as pool, \
             tc.tile_pool(name="psum", bufs=8, space="PSUM") as psp:

            WqT = cpool.tile([128, KB, NS], BF16)     # 8 MB
            WscaleB = cpool.tile([128, NS], BF16)     # 512 KB
            xnats = cpool.tile([128, NT], F32)        # un-reversed x scales

            # ---------------- W pipeline ----------------
            wtiles = {}
            wscales = {}

            def w_load(wt):
                wtile = pool.tile([128, K], F32, tag="wstage", bufs=2,
                                  name=f"wtile_{wt}")
                nc.sync.dma_start(out=wtile[:], in_=ws[wt * 128:(wt + 1) * 128])
                wtiles[wt] = wtile

            def w_absmax(wt):
                wtile = wtiles[wt]
                wabs = pool.tile([128, 1], F32, tag="sA", bufs=2, name=f"wabs_{wt}")
                nc.vector.tensor_reduce(out=wabs[:], in_=wtile[:],
                                        axis=mybir.AxisListType.X, op=ALU.max,
                                        apply_absolute_value=True)
                winv = pool.tile([128, 1], F32, tag="sB", bufs=2, name=f"winv_{wt}")
                nc.vector.reciprocal(out=winv[:], in_=wabs[:])
                winv2 = pool.tile([128, 1], F32, tag="sC", bufs=2, name=f"winv2_{wt}")
                nc.vector.tensor_scalar_mul(out=winv2[:], in0=winv[:], scalar1=224.0)
                wscale = pool.tile([128, 1], BF16, tag="sD", bufs=2,
                                   name=f"wscale_{wt}")
                nc.vector.tensor_scalar_mul(out=wscale[:], in0=wabs[:],
                                            scalar1=INV224)
                wscales[wt] = (winv2, wscale)

            def w_srow(wt):
                _, wscale = wscales[wt]
                nc.scalar.dma_start(out=wsd[0:1, wt * 128:(wt + 1) * 128],
                                  in_=wscale[:])

            def w_quant(wt):
                wtile = wtiles.pop(wt)
                winv2, _ = wscales[wt]
                wq = pool.tile([128, K], FP8, tag="wq8", bufs=1, name=f"wq_{wt}")
                nc.scalar.activation(out=wq[:], in_=wtile[:], func=ACTF.Copy,
                                     scale=winv2[:])
                wtiles[wt] = wq   # now holds the quantized tile

            def w_xpose(wt):
                wq = wtiles.pop(wt)
                wdma.dma_start_transpose(WqT[:, :, wt * 128:(wt + 1) * 128],
                                         wq[:].bitcast(BF16))

            def bcast(c):
                cs, ce = c * 512, (c + 1) * 512
                src = wsd[0:1, cs:ce]
                src = dataclasses.replace(src, ap=[[0, 128]] + list(src.ap[1:]))
                nc.scalar.dma_start(out=WscaleB[:, cs:ce], in_=src)

            # ---------------- X pipeline ----------------
            xtiles = {}
            xthrs = {}
            xsqs = {}
            xqs = {}
            xqts = {}

            def x_load(mt):
                xt = pool.tile([128, K], F32, tag="xstage", bufs=2, name=f"xt_{mt}")
                nc.sync.dma_start(out=xt[:], in_=xs[mt * 128:(mt + 1) * 128])
                xtiles[mt] = xt

            def x_tree(mt):
                xt = xtiles[mt]
                x2 = xt[:].rearrange("p (g two) -> p g two", two=2)
                # relu folded into the pair max: pr = max(max(a,0), b) [DVE]
                pr = pool.tile([128, K // 2], F32, tag="pr", bufs=1, name=f"pr_{mt}")
                nc.vector.scalar_tensor_tensor(out=pr[:], in0=x2[:, :, 0],
                                               scalar=0.0, in1=x2[:, :, 1],
                                               op0=ALU.max, op1=ALU.max)
                # raw pair min path on Pool (negatives lose to u1 >= 0 anyway)
                qs = pool.tile([128, K // 2], F32, tag="qs", bufs=1, name=f"qs_{mt}")
                nc.vector.tensor_tensor(out=qs[:], in0=x2[:, :, 0], in1=x2[:, :, 1],
                                        op=ALU.min)
                pr2 = pr[:].rearrange("p (g two) -> p g two", two=2)
                qs2 = qs[:].rearrange("p (g two) -> p g two", two=2)
                u1 = pool.tile([128, K // 4], F32, tag="u1", bufs=1, name=f"u1_{mt}")
                nc.vector.tensor_tensor(out=u1[:], in0=pr2[:, :, 0], in1=pr2[:, :, 1],
                                        op=ALU.min)
                thr = pool.tile([128, K // 4], F32, tag="thr", bufs=2, name=f"thr_{mt}")
                nc.vector.tensor_tensor(out=thr[:], in0=qs2[:, :, 0], in1=qs2[:, :, 1],
                                        op=ALU.max)
                nc.vector.tensor_tensor(out=thr[:], in0=thr[:], in1=u1[:],
                                        op=ALU.max)
                xthrs[mt] = (pr, thr)

            def x_rmax(mt):
                pr, _ = xthrs[mt]
                rmax = pool.tile([128, 1], F32, tag="sE", bufs=2, name=f"rmax_{mt}")
                nc.vector.tensor_reduce(out=rmax[:], in_=pr[:],
                                        axis=mybir.AxisListType.X, op=ALU.max)
                return rmax

            def x_scales(mt, rmax):
                rm2 = pool.tile([128, 1], F32, tag="sF", bufs=2, name=f"rm2_{mt}")
                nc.vector.tensor_scalar_max(out=rm2[:], in0=rmax[:], scalar1=1e-5)
                rrec = pool.tile([128, 1], F32, tag="sG", bufs=2, name=f"rrec_{mt}")
                nc.vector.reciprocal(out=rrec[:], in_=rm2[:])
                sq = pool.tile([128, 1], F32, tag="sH", bufs=2, name=f"sq_{mt}")
                nc.vector.tensor_scalar_mul(out=sq[:], in0=rrec[:], scalar1=SQRT224)
                xsc = pool.tile([128, 1], F32, tag="sI", bufs=2, name=f"xsc_{mt}")
                nc.vector.tensor_tensor(out=xsc[:], in0=rmax[:], in1=rmax[:],
                                        op=ALU.mult)
                xsc2 = pool.tile([128, 1], F32, tag="sJ", bufs=2, name=f"xsc2_{mt}")
                nc.vector.tensor_scalar_mul(out=xsc2[:], in0=xsc[:], scalar1=INV224)
                xsh = pool.tile([128, 1], F32, tag="sK", bufs=2, name=f"xsh_{mt}")
                nc.vector.stream_shuffle(out=xsh[:], in_=xsc2[:], mask=rev32)
                xsqs[mt] = sq
                return xsh

            def x_nat(mt, xsh):
                # un-reverse across the four 32-partition blocks
                for q in range(4):
                    nc.scalar.dma_start(out=xnats[32 * (3 - q):32 * (4 - q), mt:mt + 1],
                                      in_=xsh[32 * q:32 * (q + 1)])

            def x_finish(mt):
                xt = xtiles.pop(mt)
                _, thr = xthrs.pop(mt)
                mask = pool.tile([128, K], FP8, tag="mask", bufs=1, name=f"mask_{mt}")
                x4 = xt[:].rearrange("p (g four) -> p g four", four=4)
                m4 = mask[:].rearrange("p (g four) -> p g four", four=4)
                tb = thr[:].rearrange("p (g one) -> p g one", one=1)
                tb = dataclasses.replace(tb, ap=[tb.ap[0], tb.ap[1], [0, 4]])
                nc.vector.tensor_tensor(out=m4[:], in0=x4[:], in1=tb, op=ALU.is_ge)
                nc.vector.tensor_tensor(out=xt[:], in0=xt[:], in1=mask[:],
                                        op=ALU.mult)
                sq = xsqs.pop(mt)
                xq = pool.tile([128, K], FP8, tag="xq8", bufs=1, name=f"xq_{mt}")
                nc.scalar.activation(out=xq[:], in_=xt[:], func=ACTF.Square,
                                     scale=sq[:])
                xqs[mt] = xq

            def x_xpose(mt):
                xq = xqs.pop(mt)
                xqT = pool.tile([128, KB, 128], BF16, tag="xqT", bufs=XQT_BUFS,
                                name=f"xqT_{mt}")
                nc.scalar.dma_start_transpose(xqT[:], xq[:].bitcast(BF16))
                xqts[mt] = xqT

            # ---------------- matmul / dequant / store ----------------
            accs = {}

            def mm(mt, ch):
                if mt not in xqts:
                    x_xpose(mt)
                xqT = xqts[mt]
                acc = psp.tile([128, 512], F32, tag="acc", name=f"acc_{mt}_{ch}")
                wq8 = WqT[:].bitcast(FP8)   # [128, KB, 2*NS]
                xq8 = xqT[:].bitcast(FP8)   # [128, KB, 256]
                for blk in range(KB):
                    lhs = xq8[:, blk, :]
                    rhs = wq8[:, blk, ch * 1024:(ch + 1) * 1024].rearrange(
                        "p (n two) -> p two n", two=2)
                    nc.tensor.matmul(acc[:], lhs, rhs,
                                     start=(blk == 0), stop=(blk == KB - 1),
                                     perf_mode=mybir.MatmulPerfMode.DoubleRowSwInterleave)
                accs[(mt, ch)] = acc
                if ch == NCH - 1:
                    xqts.pop(mt)  # last reader emitted; frees the xqT slot

            def dq_store(mt, ch):
                acc = accs.pop((mt, ch))
                # acc * xnat on ACT (per-partition scale), then a packed-bf16
                # 2x multiply by the broadcast W scales on DVE
                dqt = pool.tile([128, 512], BF16, tag="dqt", bufs=2,
                                name=f"dqt_{mt}_{ch}")
                nc.scalar.activation(out=dqt[:], in_=acc[:], func=ACTF.Copy,
                                     scale=xnats[:, mt:mt + 1])
                ost = pool.tile([128, 512], BF16, tag="ost", bufs=2,
                                name=f"ost_{mt}_{ch}")
                nc.vector.tensor_tensor(
                    out=ost[:], in0=dqt[:],
                    in1=WscaleB[:, ch * 512:(ch + 1) * 512], op=ALU.mult)
                nc.scalar.dma_start(
                    out=out[mt * 128:(mt + 1) * 128, ch * 512:(ch + 1) * 512],
                    in_=ost[:])

            # ---------------- emission schedule ----------------
            pe_list = _pe_order()
            pe_pos = 0
            pending_dq = []
            x_ready = set()
            ch_ready = set()

            def pump_pe(budget):
                """Emit up to `budget` matmul (t,c) pairs (in pe_list order,
                gated on emitted deps); dequants trail by one pump call so
                the DVE never head-of-line blocks on an unfinished matmul."""
                nonlocal pe_pos
                while pending_dq:
                    dq_store(*pending_dq.pop(0))
                emitted = 0
                while pe_pos < len(pe_list) and emitted < budget:
                    t, c = pe_list[pe_pos]
                    if t not in x_ready or c not in ch_ready:
                        break
                    mm(t, c)
                    pending_dq.append((t, c))
                    pe_pos += 1
                    emitted += 1

            # prologue
            w_load(0)
            x_load(0)
            w_load(1)

            for i in range(NT + 3):
                # W chain first: its DVE work has no intra-iteration deps,
                # and the sooner wq tiles transpose, the sooner PE channels
                # unlock.
                if i < WT:
                    w_absmax(i)
                    w_srow(i)
                    w_quant(i)
                    w_xpose(i)
                    if i % 4 == 3:
                        c = i // 4
                        bcast(c)
                        ch_ready.add(c)
                # X chain: finish tile i-1 (its thr/mask deps are a full
                # iteration old, so the DVE never stalls).
                if 1 <= i <= NT:
                    t = i - 1
                    x_finish(t)
                    x_ready.add(t)
                # loads last: the quants that free their staging slots are
                # already emitted, so the pure-load SP queue never waits on
                # anything further than one iteration out.
                if i + 2 < WT:
                    w_load(i + 2)
                if i + 1 < NT:
                    x_load(i + 1)
                if i < NT:
                    x_tree(i)
                    rmax = x_rmax(i)
                    xsh = x_scales(i, rmax)
                    x_nat(i, xsh)
                pump_pe(3)
            # drain remaining matmuls + dequants
            while pe_pos < len(pe_list) or pending_dq:
                prev = pe_pos
                pump_pe(4)
                if pe_pos == prev and pe_pos < len(pe_list):
                    raise RuntimeError(
                        f"pe schedule stalled at {pe_pos}: {pe_list[pe_pos]}")

    return nc




# ===========================================================================
# v3: cross-core dedup of the x/W prep via AllGather collectives.
#
# Each x row-block was sparsified+quantized on BOTH n-half cores, and each
# W row-block quantized on all FOUR m-group cores.  v3 assigns each core a
# disjoint slice of the prep work and exchanges the quantized+transposed
# fp8 tiles through DRAM AllGathers:
#   - x: core (g,h) preps m-tiles {h*8+p} of its m-group; pairs {c, c+4}
#     gather per-tile bundles (xqT bytes + the un-reversed row scale).
#     Gathered slot 0 = global tile p, slot 1 = global tile 8+p on BOTH
#     cores, so all addressing stays SPMD-static.
#   - W: core (g,h) preps global W tiles {4j+g} (stride-4 interleave), so
#     the j-th gather over the quad {4h.. } delivers exactly channel j
#     (n columns [j*512,(j+1)*512)), keeping the per-channel PE ramp.
# DMA trigger pressure: loads on SP; transposes+quants on ACT; bounces,
# reloads, stores and small copies on the GpSimd software DGE (idle
# engine).  Stores are batched per (tile, ch-pair) rows.
# ===========================================================================

XP = 8        # x tiles prepped per core
WP = 4        # w tiles prepped per core
XGROUPS = [[0, 4], [1, 5], [2, 6], [3, 7]]
WGROUPS = [[0, 1, 2, 3], [4, 5, 6, 7]]


def _pe_order_v3():
    order = []
    for p in range(XP):
        for c in range(NCH):
            order.append((p, c))
        for c in range(NCH):
            order.append((8 + p, c))
    return order


def _build_program_v3():
    nc = bass.Bass(num_devices=N_CORES)
    xs = nc.dram_tensor("xs", [XP * 128, K], F32, kind="ExternalInput")
    ws = nc.dram_tensor("ws", [WP * 128, K], F32, kind="ExternalInput")
    out = nc.dram_tensor("out", [MS, NS], BF16, kind="ExternalOutput")
    wsd = nc.dram_tensor("wsd", [1, WP * 128], BF16, kind="Internal")
    wsg = nc.dram_tensor("wsg", [4, 1, WP * 128], BF16, kind="Internal")
    wqbo = nc.dram_tensor("wqbo", [128, WP * KB * 128], BF16, kind="Internal")
    wqbi = nc.dram_tensor("wqbi", [4, 128, WP * KB * 128], BF16, kind="Internal")
    XBN = KB * 128 + 2   # xqT bytes (bf16 cols) + scale
    xqbo = [nc.dram_tensor(f"xqbo{p}", [128, XBN], BF16, kind="Internal")
            for p in range(XP)]
    xqbi = [nc.dram_tensor(f"xqbi{p}", [2, 128, XBN], BF16, kind="Internal")
            for p in range(XP)]

    rev32 = list(range(31, -1, -1))

    with TileContext(nc) as tc:
        with tc.tile_pool(name="persist", bufs=1) as cpool, \
             tc.tile_pool(name="work", bufs=1) as pool, \
             tc.tile_pool(name="psum", bufs=8, space="PSUM") as psp:

            WqT = cpool.tile([128, KB, NS], BF16)     # 8 MB
            WscaleB = cpool.tile([128, NS], BF16)     # 512 KB
            xnats = cpool.tile([128, NT], F32)        # global-tile row scales

            # ---------------- W pipeline (front-loaded, 4 own tiles) -------
            wtiles = {}
            wscales = {}

            def w_load(j):
                wtile = pool.tile([128, K], F32, tag="stage", bufs=4,
                                  name=f"wtile_{j}")
                nc.sync.dma_start(out=wtile[:], in_=ws[j * 128:(j + 1) * 128])
                wtiles[j] = wtile

            def w_prep(j):
                wtile = wtiles[j]
                wabs = pool.tile([128, 1], F32, tag="sA", bufs=2, name=f"wabs_{j}")
                nc.vector.tensor_reduce(out=wabs[:], in_=wtile[:],
                                        axis=mybir.AxisListType.X, op=ALU.max,
                                        apply_absolute_value=True)
                winv = pool.tile([128, 1], F32, tag="sB", bufs=2, name=f"winv_{j}")
                nc.vector.reciprocal(out=winv[:], in_=wabs[:])
                winv2 = pool.tile([128, 1], F32, tag="sC", bufs=2, name=f"winv2_{j}")
                nc.vector.tensor_scalar_mul(out=winv2[:], in0=winv[:], scalar1=224.0)
                wscale = pool.tile([128, 1], BF16, tag="sD", bufs=2,
                                   name=f"wscale_{j}")
                nc.vector.tensor_scalar_mul(out=wscale[:], in0=wabs[:],
                                            scalar1=INV224)
                nc.gpsimd.dma_start(out=wsd[0:1, j * 128:(j + 1) * 128],
                                    in_=wscale[:])
                wtile2 = wtiles.pop(j)
                wq = pool.tile([128, K], FP8, tag="wq8", bufs=1, name=f"wq_{j}")
                nc.scalar.activation(out=wq[:], in_=wtile2[:], func=ACTF.Copy,
                                     scale=winv2[:])
                wqt = pool.tile([128, KB, 128], BF16, tag="qto", bufs=2,
                                name=f"wqt_{j}")
                nc.scalar.dma_start_transpose(wqt[:], wq[:].bitcast(BF16))
                nc.gpsimd.dma_start(
                    out=wqbo[:, j * KB * 128:(j + 1) * KB * 128], in_=wqt[:])

            def w_cc():
                nc.gpsimd.collective_compute(
                    "AllGather", ALU.bypass, replica_groups=WGROUPS,
                    ins=[wqbo[:].opt()], outs=[wqbi[:].opt()])

            def w_reload_all():
                for j in range(WP):
                    for g2 in range(4):
                        wt = 4 * j + g2
                        nc.gpsimd.dma_start(
                            out=WqT[:, :, wt * 128:(wt + 1) * 128],
                            in_=wqbi[g2, :, j * KB * 128:(j + 1) * KB * 128]
                            .rearrange("p (kb n) -> p kb n", kb=KB))

            def w_scales_cc():
                nc.gpsimd.collective_compute(
                    "AllGather", ALU.bypass, replica_groups=WGROUPS,
                    ins=[wsd[:].opt()], outs=[wsg[:].opt()])

            def w_scales_bcast():
                for j in range(WP):
                    for g2 in range(4):
                        wt = 4 * j + g2
                        src = wsg[g2, 0:1, j * 128:(j + 1) * 128]
                        src = dataclasses.replace(
                            src, ap=[[0, 128]] + list(src.ap[1:]))
                        nc.gpsimd.dma_start(
                            out=WscaleB[:, wt * 128:(wt + 1) * 128], in_=src)

            # ---------------- X pipeline (8 own tiles + pair gathers) ------
            xtiles = {}
            xthrs = {}
            xsqs = {}
            xqts = {}

            def x_load(p):
                xt = pool.tile([128, K], F32, tag="stage", bufs=4, name=f"xt_{p}")
                nc.sync.dma_start(out=xt[:], in_=xs[p * 128:(p + 1) * 128])
                xtiles[p] = xt

            def x_tree(p):
                xt = xtiles[p]
                x2 = xt[:].rearrange("p (g two) -> p g two", two=2)
                pr = pool.tile([128, K // 2], F32, tag="pr", bufs=1, name=f"pr_{p}")
                nc.vector.scalar_tensor_tensor(out=pr[:], in0=x2[:, :, 0],
                                               scalar=0.0, in1=x2[:, :, 1],
                                               op0=ALU.max, op1=ALU.max)
                qs = pool.tile([128, K // 2], F32, tag="qs", bufs=1, name=f"qs_{p}")
                nc.vector.tensor_tensor(out=qs[:], in0=x2[:, :, 0], in1=x2[:, :, 1],
                                        op=ALU.min)
                rmax = pool.tile([128, 1], F32, tag="sE", bufs=2, name=f"rmax_{p}")
                nc.vector.tensor_reduce(out=rmax[:], in_=pr[:],
                                        axis=mybir.AxisListType.X, op=ALU.max)
                pr2 = pr[:].rearrange("p (g two) -> p g two", two=2)
                qs2 = qs[:].rearrange("p (g two) -> p g two", two=2)
                u1 = pr[:, 0:K // 4]
                nc.vector.tensor_tensor(out=u1, in0=pr2[:, :, 0], in1=pr2[:, :, 1],
                                        op=ALU.min)
                thr = qs[:, 0:K // 4]
                nc.vector.tensor_tensor(out=thr, in0=qs2[:, :, 0], in1=qs2[:, :, 1],
                                        op=ALU.max)
                nc.vector.tensor_tensor(out=thr, in0=thr, in1=u1,
                                        op=ALU.max)
                xthrs[p] = (pr, qs)
                rm2 = pool.tile([128, 1], F32, tag="sF", bufs=2, name=f"rm2_{p}")
                nc.vector.tensor_scalar_max(out=rm2[:], in0=rmax[:], scalar1=1e-5)
                rrec = pool.tile([128, 1], F32, tag="sG", bufs=2, name=f"rrec_{p}")
                nc.vector.reciprocal(out=rrec[:], in_=rm2[:])
                sq = pool.tile([128, 1], F32, tag="sH", bufs=2, name=f"sq_{p}")
                nc.vector.tensor_scalar_mul(out=sq[:], in0=rrec[:], scalar1=SQRT224)
                xsc = pool.tile([128, 1], F32, tag="sI", bufs=2, name=f"xsc_{p}")
                nc.vector.tensor_tensor(out=xsc[:], in0=rmax[:], in1=rmax[:],
                                        op=ALU.mult)
                xsc2 = pool.tile([128, 1], F32, tag="sJ", bufs=2, name=f"xsc2_{p}")
                nc.vector.tensor_scalar_mul(out=xsc2[:], in0=xsc[:], scalar1=INV224)
                xsh = pool.tile([128, 1], F32, tag="sK", bufs=2, name=f"xsh_{p}")
                nc.vector.stream_shuffle(out=xsh[:], in_=xsc2[:], mask=rev32)
                xsqs[p] = (sq, xsh)

            def x_finish(p):
                xt = xtiles.pop(p)
                _, qs = xthrs.pop(p)
                thr = qs[:, 0:K // 4]
                mask = pool.tile([128, K], FP8, tag="mask", bufs=1, name=f"mask_{p}")
                x4 = xt[:].rearrange("p (g four) -> p g four", four=4)
                m4 = mask[:].rearrange("p (g four) -> p g four", four=4)
                tb = thr.rearrange("p (g one) -> p g one", one=1)
                tb = dataclasses.replace(tb, ap=[tb.ap[0], tb.ap[1], [0, 4]])
                nc.vector.tensor_tensor(out=m4[:], in0=x4[:], in1=tb, op=ALU.is_ge)
                nc.vector.tensor_tensor(out=xt[:], in0=xt[:], in1=mask[:],
                                        op=ALU.mult)
                sq, xsh = xsqs.pop(p)
                xq = pool.tile([128, K], FP8, tag="xq8", bufs=1, name=f"xq_{p}")
                nc.scalar.activation(out=xq[:], in_=xt[:], func=ACTF.Square,
                                     scale=sq[:])
                xqto = pool.tile([128, KB, 128], BF16, tag="qto", bufs=2,
                                 name=f"xqto_{p}")
                nc.scalar.dma_start_transpose(xqto[:], xq[:].bitcast(BF16))
                # un-reverse the row scale into a compact tile, then bounce
                # bundle = [xqT bytes | scale-as-2-bf16 | pad]
                xno = pool.tile([128, 1], F32, tag="sL", bufs=2, name=f"xno_{p}")
                for q in range(4):
                    nc.gpsimd.dma_start(out=xno[32 * (3 - q):32 * (4 - q)],
                                        in_=xsh[32 * q:32 * (q + 1)])
                nc.gpsimd.dma_start(out=xqbo[p][:, 0:KB * 128], in_=xqto[:])
                nc.gpsimd.dma_start(out=xqbo[p][:, KB * 128:KB * 128 + 2],
                                    in_=xno[:].bitcast(BF16))
                nc.gpsimd.collective_compute(
                    "AllGather", ALU.bypass, replica_groups=XGROUPS,
                    ins=[xqbo[p][:].opt()], outs=[xqbi[p][:].opt()])

            def x_reload(p):
                for i in range(2):
                    t = 8 * i + p
                    xqT = pool.tile([128, KB, 128], BF16, tag="xqT", bufs=6,
                                    name=f"xqT_{t}")
                    nc.gpsimd.dma_start(
                        out=xqT[:],
                        in_=xqbi[p][i, :, 0:KB * 128].rearrange(
                            "p (kb n) -> p kb n", kb=KB))
                    xqts[t] = xqT
                    nc.gpsimd.dma_start(
                        out=xnats[:, t:t + 1],
                        in_=xqbi[p][i, :, KB * 128:KB * 128 + 2].bitcast(F32))

            # ---------------- matmul / dequant / store ---------------------
            accs = {}
            osts = {}

            def mm(t, ch):
                xqT = xqts[t]
                acc = psp.tile([128, 512], F32, tag="acc", name=f"acc_{t}_{ch}")
                wq8 = WqT[:].bitcast(FP8)
                xq8 = xqT[:].bitcast(FP8)
                for blk in range(KB):
                    lhs = xq8[:, blk, :]
                    rhs = wq8[:, blk, ch * 1024:(ch + 1) * 1024].rearrange(
                        "p (n two) -> p two n", two=2)
                    nc.tensor.matmul(acc[:], lhs, rhs,
                                     start=(blk == 0), stop=(blk == KB - 1),
                                     perf_mode=mybir.MatmulPerfMode.DoubleRowSwInterleave)
                accs[(t, ch)] = acc
                if ch == NCH - 1:
                    xqts.pop(t)

            def dq(t, ch):
                acc = accs.pop((t, ch))
                dqt = pool.tile([128, 512], BF16, tag="dqt", bufs=1,
                                name=f"dqt_{t}_{ch}")
                nc.scalar.activation(out=dqt[:], in_=acc[:], func=ACTF.Copy,
                                     scale=xnats[:, t:t + 1])
                if t not in osts:
                    osts[t] = pool.tile([128, NS], BF16, tag="ost", bufs=2,
                                        name=f"ost_{t}")
                ost = osts[t]
                nc.vector.tensor_tensor(
                    out=ost[:, ch * 512:(ch + 1) * 512], in0=dqt[:],
                    in1=WscaleB[:, ch * 512:(ch + 1) * 512], op=ALU.mult)
                if ch == NCH - 1:
                    nc.gpsimd.dma_start(out=out[t * 128:(t + 1) * 128], in_=ost[:])
                    osts.pop(t)

            # ---------------- emission -------------------------------------
            pe_list = _pe_order_v3()
            pe_pos = 0
            pending_dq = []
            x_ready = set()
            ch_ready = set()

            def pump_pe(budget):
                nonlocal pe_pos
                while pending_dq:
                    dq(*pending_dq.pop(0))
                emitted = 0
                while pe_pos < len(pe_list) and emitted < budget:
                    t, c = pe_list[pe_pos]
                    if t not in x_ready or c not in ch_ready:
                        break
                    mm(t, c)
                    pending_dq.append((t, c))
                    pe_pos += 1
                    emitted += 1

            # W phase: all four own tiles, fully pipelined through the
            # gathers; x loads stream behind the W loads on SP.
            w_load(0)
            w_load(1)
            x_load(0)
            for j in range(WP):
                w_prep(j)
                if j + 2 < WP:
                    w_load(j + 2)
                else:
                    x_load(1 + (j + 2 - WP))
            w_cc()
            w_scales_cc()

            for i in range(XP + 3):
                if 1 <= i <= XP:
                    p = i - 1
                    x_finish(p)
                if i == 1:
                    # the W gather completed while x pair 0 was prepped;
                    # its reloads slot in behind the first x gather
                    w_reload_all()
                    w_scales_bcast()
                    for c in range(NCH):
                        ch_ready.add(c)
                if 2 <= i <= XP + 1:
                    p = i - 2
                    x_reload(p)
                    x_ready.add(p)
                    x_ready.add(8 + p)
                if i + 3 < XP:
                    x_load(i + 3)
                if i < XP:
                    x_tree(i)
                pump_pe(4)
            while pe_pos < len(pe_list) or pending_dq:
                prev = pe_pos
                pump_pe(6)
                if pe_pos == prev and pe_pos < len(pe_list):
                    raise RuntimeError(
                        f"pe schedule stalled at {pe_pos}: {pe_list[pe_pos]}")

    return nc


def _run_v3(x, W, trace=False):
    x = np.ascontiguousarray(x, dtype=np.float32)
    W = np.ascontiguousarray(W, dtype=np.float32)
    nc = _get_nc()
    in_maps = []
    for c in range(N_CORES):
        g, h = c % MG, c // MG
        xg = x[g * MS:(g + 1) * MS].reshape(NT, 128, K)[:, ::-1, :]
        xsh = xg[h * XP:(h + 1) * XP].reshape(XP * 128, K)
        wrows = np.concatenate(
            [W[h * NS + (4 * j + g) * 128: h * NS + (4 * j + g + 1) * 128]
             for j in range(WP)], axis=0)
        in_maps.append({
            "xs": np.ascontiguousarray(xsh),
            "ws": np.ascontiguousarray(wrows),
        })
    res = run_bass_kernel_spmd(nc, in_maps, core_ids=list(range(N_CORES)),
                               trace=trace)
    outf = np.empty((M, NW), dtype=ml_dtypes.bfloat16)
    for c in range(N_CORES):
        g, h = c % MG, c // MG
        outf[g * MS:(g + 1) * MS, h * NS:(h + 1) * NS] = res.results[c]["out"]
    return outf, res

_cached_nc = None
KV3 = os.environ.get("KV3", "1") == "1"


def _get_nc():
    global _cached_nc
    if _cached_nc is None:
        _cached_nc = _build_program_v3() if KV3 else _build_program()
    return _cached_nc


def _run(x, W, trace=False):
    if KV3:
        return _run_v3(x, W, trace=trace)
    x = np.ascontiguousarray(x, dtype=np.float32)
    W = np.ascontiguousarray(W, dtype=np.float32)
    assert x.shape == (M, K) and W.shape == (NW, K)
    nc = _get_nc()
    in_maps = []
    for c in range(N_CORES):
        g, h = c % MG, c // MG
        xsh = x[g * MS:(g + 1) * MS].reshape(NT, 128, K)[:, ::-1, :].reshape(MS, K)
        in_maps.append({
            "xs": np.ascontiguousarray(xsh),
            "ws": W[h * NS:(h + 1) * NS],
        })
    res = run_bass_kernel_spmd(nc, in_maps, core_ids=list(range(N_CORES)),
                               trace=trace)
    outf = np.empty((M, NW), dtype=ml_dtypes.bfloat16)
    for c in range(N_CORES):
        g, h = c % MG, c // MG
        outf[g * MS:(g + 1) * MS, h * NS:(h + 1) * NS] = res.results[c]["out"]
    return outf, res


def kernel(x, W):
    out, _ = _run(x, W, trace=False)
    return out


# revision 19
# speedup vs baseline: 1.2291x; 1.1317x over previous
"""FP8 semi-sparse activation linear kernel for Trainium2 (8 NeuronCores).

Computes: rowwise-fp8-quant(2:4-sparsify(relu(x)^2)) @ rowwise-fp8-quant(W).T -> bf16

Sharding: x rows split 4 ways (m-groups), W rows (= out cols) split 2 ways
(n-halves); core c handles m-group c % 4, n-half c // 4.

Key implementation notes:
  - TRN fp8e4 max is +-240 (vs OCP e4m3fn +-448).  We quantize to +-224
    (scale' = 2*scale_ref); powers of two commute with RNE so the fp8
    rounding grid matches the reference exactly (sans the denormal tail,
    which is ~2^-18 relative -- irrelevant).
  - The 2:4 sparsify runs on r = relu(x) (monotonic under squaring); the
    square is fused into the fp8 quantization via ACT's Square activation
    (out = Square(r * sqrt(inv))), which hardware evaluates exactly.
  - Transposes (K onto partitions) use the DMA xbar on bf16-bitcast fp8
    pairs: one dma_start_transpose per [128, 4096-fp8] tile.  Resulting
    layout: partition p, k-block b holds k = 256*b + 2*p + {0,1} as
    adjacent bytes.
  - The matmul runs in fp8 DoubleRow (2x) mode: the moving operand uses a
    [p, 2, n] AP (pair step 1 byte -- legal on the MM side), the
    stationary operand uses DoubleRowSwInterleave which expects adjacent
    A/B byte pairs with columns reversed; we pre-reverse x rows per
    128-tile on the host so PSUM rows come out in natural order.
  - Per-row x scales are computed on reversed rows; they are un-reversed
    on chip with a stream_shuffle (reverse within 32) + 4 partition-block
    DMA copies.
"""
import sys
import os
import dataclasses

sys.path.insert(0, "/opt/trn_rl_repo")

import numpy as np
import ml_dtypes

import concourse.bass as bass
import concourse.mybir as mybir
from concourse.tile import TileContext
from concourse.bass_utils import run_bass_kernel_spmd

# ---------------------------------------------------------------------------
# Workaround: this environment's walrus rejects instructions with more than
# a couple of sync-wait conditions ("Too many sync wait commands").  Split
# excess waits onto NoOp instructions inserted before the offender.
import orjson as _orjson

_orig_to_json_bytes = bass.Bass.to_json_bytes
_LIMIT_DEFAULT = 1
_ws_counter = [0]


def _split_waits(doc):
    for fn in doc.get("functions", []):
        for blk in fn.get("blocks", []):
            insts = blk.get("instructions")
            if not insts:
                continue
            out = []
            changed = False
            for ins in insts:
                si = ins.get("sync_info")
                if si:
                    waits = si.get("on_wait") or []
                    if len(waits) > _LIMIT_DEFAULT:
                        excess = waits[:-_LIMIT_DEFAULT]
                        keep = waits[-_LIMIT_DEFAULT:]
                        for i in range(0, len(excess), _LIMIT_DEFAULT):
                            _ws_counter[0] += 1
                            out.append({
                                "name": f"I-waitsplit-{_ws_counter[0]}",
                                "engine": ins["engine"],
                                "opcode": "NoOp",
                                "ins": [],
                                "outs": [],
                                "sync_info": {
                                    "on_wait": excess[i:i + _LIMIT_DEFAULT],
                                    "on_update": [],
                                },
                            })
                        si["on_wait"] = keep
                        changed = True
                out.append(ins)
            if changed:
                blk["instructions"] = out
    return doc


def _patched_to_json_bytes(self):
    return _orjson.dumps(_split_waits(_orjson.loads(_orig_to_json_bytes(self))))


bass.Bass.to_json_bytes = _patched_to_json_bytes
# ---------------------------------------------------------------------------

F32 = mybir.dt.float32
FP8 = mybir.dt.float8e4
BF16 = mybir.dt.bfloat16
ALU = mybir.AluOpType
ACTF = mybir.ActivationFunctionType

M, K, NW = 8192, 4096, 4096
N_CORES = 8
MG, NH = 4, 2                  # m-groups x n-halves
MS, NS = M // MG, NW // NH     # 2048 x 2048 per-core output shard
NT = MS // 128                 # 16 x-tiles
WT = NS // 128                 # 16 w-tiles
KB = K // 256                  # 16 k-blocks of 256
SQRT224 = float(np.float32(np.sqrt(np.float32(224.0))))
INV224 = float(np.float32(1.0) / np.float32(224.0))


def _build_program():
    nc = bass.Bass()
    xs = nc.dram_tensor("xs", [MS, K], F32, kind="ExternalInput")
    ws = nc.dram_tensor("ws", [NS, K], F32, kind="ExternalInput")
    out = nc.dram_tensor("out", [MS, NS], BF16, kind="ExternalOutput")
    wsd = nc.dram_tensor("wsd", [128, NS], F32, kind="Internal")

    rev32 = list(range(31, -1, -1))

    with TileContext(nc) as tc:
        with tc.tile_pool(name="persist", bufs=1) as cpool, \
             tc.tile_pool(name="work", bufs=1) as pool, \
             tc.tile_pool(name="psum", bufs=8, space="PSUM") as psp:

            WqT = cpool.tile([128, KB, NS], BF16)     # 8 MB
            WscaleB = cpool.tile([128, NS], F32)      # 1 MB
            wsrow = cpool.tile([1, NS], F32)

            # NOTE: every DMA (loads, xbar transposes, stores, small copies)
            # is issued from nc.sync -- concurrent DMA on another ring
            # corrupts in-flight xbar transposes (HW bug, verified).

            wtiles = {}

            def w_load(wt):
                wtile = pool.tile([128, K], F32, tag="wtile", bufs=2,
                                  name=f"wtile_{wt}")
                nc.sync.dma_start(out=wtile[:], in_=ws[wt * 128:(wt + 1) * 128])
                wtiles[wt] = wtile

            def w_rest(wt):
                wtile = wtiles.pop(wt)
                wabs = pool.tile([128, 1], F32, tag="sA", bufs=2, name=f"wabs_{wt}")
                nc.vector.tensor_reduce(out=wabs[:], in_=wtile[:],
                                        axis=mybir.AxisListType.X, op=ALU.max,
                                        apply_absolute_value=True)
                winv = pool.tile([128, 1], F32, tag="sB", bufs=2, name=f"winv_{wt}")
                nc.vector.reciprocal(out=winv[:], in_=wabs[:])
                winv2 = pool.tile([128, 1], F32, tag="sC", bufs=2, name=f"winv2_{wt}")
                nc.vector.tensor_scalar_mul(out=winv2[:], in0=winv[:], scalar1=224.0)
                wscale = pool.tile([128, 1], F32, tag="sD", bufs=2, name=f"wscale_{wt}")
                nc.vector.tensor_scalar_mul(out=wscale[:], in0=wabs[:], scalar1=INV224)
                nc.sync.dma_start(out=wsrow[0:1, wt * 128:(wt + 1) * 128],
                                  in_=wscale[:])
                wq = pool.tile([128, K], FP8, tag="wq8", bufs=1, name=f"wq_{wt}")
                nc.scalar.activation(out=wq[:], in_=wtile[:], func=ACTF.Copy,
                                     scale=winv2[:])
                nc.sync.dma_start_transpose(WqT[:, :, wt * 128:(wt + 1) * 128],
                                            wq[:].bitcast(BF16))

            xts = {}
            xqs = {}
            xqts = {}
            xshs = {}
            xnats = {}

            def x_pre(mt):
                xt = pool.tile([128, K], F32, tag="xt", bufs=2, name=f"xt_{mt}")
                nc.sync.dma_start(out=xt[:], in_=xs[mt * 128:(mt + 1) * 128])
                xts[mt] = xt

            def x_post(mt):
                r = xts[mt]
                r2 = r[:].rearrange("p (g two) -> p g two", two=2)
                pr = pool.tile([128, K // 2], F32, tag="pr", name=f"pr_{mt}")
                qs = pool.tile([128, K // 2], F32, tag="qs", name=f"qs_{mt}")
                nc.vector.scalar_tensor_tensor(out=pr[:], in0=r2[:, :, 0],
                                               scalar=0.0, in1=r2[:, :, 1],
                                               op0=ALU.max, op1=ALU.max)
                nc.vector.tensor_tensor(out=qs[:], in0=r2[:, :, 0], in1=r2[:, :, 1], op=ALU.min)

                rmax = pool.tile([128, 1], F32, tag="sE", bufs=2, name=f"rmax_{mt}")
                nc.vector.tensor_reduce(out=rmax[:], in_=pr[:],
                                        axis=mybir.AxisListType.X, op=ALU.max)

                pr2 = pr[:].rearrange("p (g two) -> p g two", two=2)
                qs2 = qs[:].rearrange("p (g two) -> p g two", two=2)
                u1t = pool.tile([128, K // 4], F32, tag="u1", name=f"u1_{mt}")
                u2t = pool.tile([128, K // 4], F32, tag="u2", name=f"u2_{mt}")
                thrt = pool.tile([128, K // 4], F32, tag="thr", name=f"thr_{mt}")
                nc.vector.tensor_tensor(out=u1t[:], in0=pr2[:, :, 0], in1=pr2[:, :, 1], op=ALU.min)
                nc.vector.tensor_tensor(out=u2t[:], in0=qs2[:, :, 0], in1=qs2[:, :, 1], op=ALU.max)
                nc.vector.tensor_tensor(out=thrt[:], in0=u1t[:], in1=u2t[:], op=ALU.max)
                thr = thrt[:]

                mask = pool.tile([128, K], FP8, tag="qs", name=f"mask_{mt}")
                r4 = r[:].rearrange("p (g four) -> p g four", four=4)
                m4 = mask[:].rearrange("p (g four) -> p g four", four=4)
                tb = thr.rearrange("p (g one) -> p g one", one=1)
                tb = dataclasses.replace(tb, ap=[tb.ap[0], tb.ap[1], [0, 4]])
                nc.vector.tensor_tensor(out=m4[:], in0=r4[:], in1=tb, op=ALU.is_ge)
                nc.vector.tensor_tensor(out=r[:], in0=r[:], in1=mask[:], op=ALU.mult)

                rm2 = pool.tile([128, 1], F32, tag="sF", bufs=2, name=f"rm2_{mt}")
                nc.vector.tensor_scalar_max(out=rm2[:], in0=rmax[:], scalar1=1e-5)
                rrec = pool.tile([128, 1], F32, tag="sG", bufs=2, name=f"rrec_{mt}")
                nc.vector.reciprocal(out=rrec[:], in_=rm2[:])
                sq = pool.tile([128, 1], F32, tag="sH", bufs=2, name=f"sq_{mt}")
                nc.vector.tensor_scalar_mul(out=sq[:], in0=rrec[:], scalar1=SQRT224)
                xsc = pool.tile([128, 1], F32, tag="sI", bufs=2, name=f"xsc_{mt}")
                nc.vector.tensor_tensor(out=xsc[:], in0=rmax[:], in1=rmax[:], op=ALU.mult)
                xsc2 = pool.tile([128, 1], F32, tag="sJ", bufs=2, name=f"xsc2_{mt}")
                nc.vector.tensor_scalar_mul(out=xsc2[:], in0=xsc[:], scalar1=INV224)
                xsh = pool.tile([128, 1], F32, tag="sK", bufs=4, name=f"xsh_{mt}")
                nc.vector.stream_shuffle(out=xsh[:], in_=xsc2[:], mask=rev32)
                xshs[mt] = xsh

                xq = pool.tile([128, K], FP8, tag="xq8", bufs=3, name=f"xq_{mt}")
                nc.scalar.activation(out=xq[:], in_=r[:], func=ACTF.Square, scale=sq[:])
                xqs[mt] = xq

            def mm_mm(mt):
                xq = xqs[mt]
                xqT = pool.tile([128, KB, 128], BF16, tag="xqT", bufs=4,
                                name=f"xqT_{mt}")
                nc.sync.dma_start_transpose(xqT[:], xq[:].bitcast(BF16))
                xqts[mt] = xqT
                xnat = pool.tile([128, 1], F32, tag="sL", bufs=2, name=f"xnat_{mt}")
                xsh = xshs[mt]
                for q in range(4):
                    nc.sync.dma_start(out=xnat[32 * (3 - q):32 * (4 - q)],
                                      in_=xsh[32 * q:32 * (q + 1)])
                xnats[mt] = xnat
                accs = [psp.tile([128, 512], F32, tag="acc", name=f"acc_{mt}_{ch}")
                        for ch in range(4)]
                wq8 = WqT[:].bitcast(FP8)  # [128, KB, 2*NS]
                xq8 = xqT[:].bitcast(FP8)  # [128, KB, 256]
                for ch in range(4):
                    for blk in range(KB):
                        lhs = xq8[:, blk, :]
                        rhs = wq8[:, blk, ch * 1024:(ch + 1) * 1024].rearrange(
                            "p (n two) -> p two n", two=2)
                        nc.tensor.matmul(accs[ch][:], lhs, rhs,
                                         start=(blk == 0), stop=(blk == KB - 1),
                                         perf_mode=mybir.MatmulPerfMode.DoubleRowSwInterleave)
                return accs

            def dequant(mt, accs):
                xnat = xnats[mt]
                ost = pool.tile([128, NS], BF16, tag="wq8", bufs=1, name=f"ost_{mt}")
                for ch in range(4):
                    nc.vector.scalar_tensor_tensor(
                        out=ost[:, ch * 512:(ch + 1) * 512],
                        in0=accs[ch][:], scalar=xnat[:],
                        in1=WscaleB[:, ch * 512:(ch + 1) * 512],
                        op0=ALU.mult, op1=ALU.mult)
                nc.sync.dma_start(out=out[mt * 128:(mt + 1) * 128], in_=ost[:])

            # ---- emission schedule (single DMA ring, software-pipelined) ----
            def bcast_all():
                # replicate wsrow [1, NS] across 128 partitions via a DRAM
                # doubling chain (PE-free, ring-only)
                nc.sync.dma_start(out=wsd[0:1], in_=wsrow[:])
                k = 1
                while k < 128:
                    nc.sync.dma_start(out=wsd[k:2 * k], in_=wsd[0:k])
                    k *= 2
                nc.sync.dma_start(out=WscaleB[:], in_=wsd[:])

            # W pipeline: loads run 2 ahead of the compute+transpose tail so the
            # single DMA ring never stalls a load behind a compute-gated
            # transpose.  x tiles 0/1 load early to warm the X pipeline.
            w_load(0)
            w_load(1)
            x_pre(0)
            x_pre(1)
            for wt in range(2, WT):
                w_load(wt)
                w_rest(wt - 2)
                if wt == 10:
                    x_post(0)
                    x_pre(2)
                if wt == 13:
                    x_post(1)
                    x_pre(3)
            w_rest(WT - 2)
            w_rest(WT - 1)
            bcast_all()
            x_post(2)
            x_pre(4)

            pending = {}
            for mt in range(NT):
                pending[mt] = mm_mm(mt)
                if mt + 3 < NT:
                    x_post(mt + 3)
                if mt + 5 < NT:
                    x_pre(mt + 5)
                if mt >= 1:
                    dequant(mt - 1, pending.pop(mt - 1))
            dequant(NT - 1, pending.pop(NT - 1))

    return nc


_cached_nc = None


def _get_nc():
    global _cached_nc
    if _cached_nc is None:
        _cached_nc = _build_program()
    return _cached_nc


def _run(x, W, trace=False):
    x = np.ascontiguousarray(x, dtype=np.float32)
    W = np.ascontiguousarray(W, dtype=np.float32)
    assert x.shape == (M, K) and W.shape == (NW, K)
    nc = _get_nc()
    in_maps = []
    for c in range(N_CORES):
        g, h = c % MG, c // MG
        xsh = x[g * MS:(g + 1) * MS].reshape(NT, 128, K)[:, ::-1, :].reshape(MS, K)
        in_maps.append({
            "xs": np.ascontiguousarray(xsh),
            "ws": W[h * NS:(h + 1) * NS],
        })
    res = run_bass_kernel_spmd(nc, in_maps, core_ids=list(range(N_CORES)),
                               trace=trace)
    outf = np.empty((M, NW), dtype=ml_dtypes.bfloat16)
    for c in range(N_CORES):
        g, h = c % MG, c // MG
        outf[g * MS:(g + 1) * MS, h * NS:(h + 1) * NS] = res.results[c]["out"]
    return outf, res


def kernel(x, W):
    out, _ = _run(x, W, trace=False)
    return out



# revision 21
# speedup vs baseline: 1.2665x; 1.0304x over previous
"""FP8 semi-sparse activation linear kernel for Trainium2 (8 NeuronCores).

Computes: rowwise-fp8-quant(2:4-sparsify(relu(x)^2)) @ rowwise-fp8-quant(W).T -> bf16

Sharding: x rows split 4 ways (m-groups), W rows (= out cols) split 2 ways
(n-halves); core c handles m-group c % 4, n-half c // 4.

Key implementation notes:
  - TRN fp8e4 max is +-240 (vs OCP e4m3fn +-448).  We quantize to +-224
    (scale' = 2*scale_ref); powers of two commute with RNE so the fp8
    rounding grid matches the reference exactly (sans the denormal tail,
    which is ~2^-18 relative -- irrelevant).
  - The 2:4 sparsify runs on r = relu(x) (monotonic under squaring); the
    square is fused into the fp8 quantization via ACT's Square activation
    (out = Square(r * sqrt(inv))), which hardware evaluates exactly.
  - Transposes (K onto partitions) use the DMA xbar on bf16-bitcast fp8
    pairs: one dma_start_transpose per [128, 4096-fp8] tile.  Resulting
    layout: partition p, k-block b holds k = 256*b + 2*p + {0,1} as
    adjacent bytes.
  - The matmul runs in fp8 DoubleRow (2x) mode: the moving operand uses a
    [p, 2, n] AP (pair step 1 byte -- legal on the MM side), the
    stationary operand uses DoubleRowSwInterleave which expects adjacent
    A/B byte pairs with columns reversed; we pre-reverse x rows per
    128-tile on the host so PSUM rows come out in natural order.
  - Per-row x scales are computed on reversed rows; they are un-reversed
    on chip with a stream_shuffle (reverse within 32) + 4 partition-block
    DMA copies.
"""
import sys
import os
import dataclasses

sys.path.insert(0, "/opt/trn_rl_repo")

import numpy as np
import ml_dtypes

import concourse.bass as bass
import concourse.mybir as mybir
from concourse.tile import TileContext
from concourse.bass_utils import run_bass_kernel_spmd

# ---------------------------------------------------------------------------
# Workaround: this environment's walrus rejects instructions with more than
# a couple of sync-wait conditions ("Too many sync wait commands").  Split
# excess waits onto NoOp instructions inserted before the offender.
import orjson as _orjson

_orig_to_json_bytes = bass.Bass.to_json_bytes
_LIMIT_DEFAULT = 1
_ws_counter = [0]


def _split_waits(doc):
    for fn in doc.get("functions", []):
        for blk in fn.get("blocks", []):
            insts = blk.get("instructions")
            if not insts:
                continue
            out = []
            changed = False
            for ins in insts:
                si = ins.get("sync_info")
                if si:
                    waits = si.get("on_wait") or []
                    if len(waits) > _LIMIT_DEFAULT:
                        excess = waits[:-_LIMIT_DEFAULT]
                        keep = waits[-_LIMIT_DEFAULT:]
                        for i in range(0, len(excess), _LIMIT_DEFAULT):
                            _ws_counter[0] += 1
                            out.append({
                                "name": f"I-waitsplit-{_ws_counter[0]}",
                                "engine": ins["engine"],
                                "opcode": "NoOp",
                                "ins": [],
                                "outs": [],
                                "sync_info": {
                                    "on_wait": excess[i:i + _LIMIT_DEFAULT],
                                    "on_update": [],
                                },
                            })
                        si["on_wait"] = keep
                        changed = True
                out.append(ins)
            if changed:
                blk["instructions"] = out
    return doc


def _patched_to_json_bytes(self):
    return _orjson.dumps(_split_waits(_orjson.loads(_orig_to_json_bytes(self))))


bass.Bass.to_json_bytes = _patched_to_json_bytes
# ---------------------------------------------------------------------------

F32 = mybir.dt.float32
FP8 = mybir.dt.float8e4
BF16 = mybir.dt.bfloat16
ALU = mybir.AluOpType
ACTF = mybir.ActivationFunctionType

M, K, NW = 8192, 4096, 4096
N_CORES = 8
MG, NH = 4, 2                  # m-groups x n-halves
MS, NS = M // MG, NW // NH     # 2048 x 2048 per-core output shard
NT = MS // 128                 # 16 x-tiles
WT = NS // 128                 # 16 w-tiles
KB = K // 256                  # 16 k-blocks of 256
SQRT224 = float(np.float32(np.sqrt(np.float32(224.0))))
INV224 = float(np.float32(1.0) / np.float32(224.0))


def _build_program():
    nc = bass.Bass()
    xs = nc.dram_tensor("xs", [MS, K], F32, kind="ExternalInput")
    ws = nc.dram_tensor("ws", [NS, K], F32, kind="ExternalInput")
    out = nc.dram_tensor("out", [MS, NS], BF16, kind="ExternalOutput")
    wsd = nc.dram_tensor("wsd", [128, NS], F32, kind="Internal")

    rev32 = list(range(31, -1, -1))

    with TileContext(nc) as tc:
        with tc.tile_pool(name="persist", bufs=1) as cpool, \
             tc.tile_pool(name="work", bufs=1) as pool, \
             tc.tile_pool(name="psum", bufs=8, space="PSUM") as psp:

            WqT = cpool.tile([128, KB, NS], BF16)     # 8 MB
            WscaleB = cpool.tile([128, NS], F32)      # 1 MB
            wsrow = cpool.tile([1, NS], F32)

            # NOTE: every DMA (loads, xbar transposes, stores, small copies)
            # is issued from nc.sync -- concurrent DMA on another ring
            # corrupts in-flight xbar transposes (HW bug, verified).

            wtiles = {}

            def w_load(wt):
                wtile = pool.tile([128, K], F32, tag="wtile", bufs=2,
                                  name=f"wtile_{wt}")
                nc.sync.dma_start(out=wtile[:], in_=ws[wt * 128:(wt + 1) * 128])
                wtiles[wt] = wtile

            def w_rest(wt):
                wtile = wtiles.pop(wt)
                wabs = pool.tile([128, 1], F32, tag="sA", bufs=2, name=f"wabs_{wt}")
                nc.vector.tensor_reduce(out=wabs[:], in_=wtile[:],
                                        axis=mybir.AxisListType.X, op=ALU.max,
                                        apply_absolute_value=True)
                winv = pool.tile([128, 1], F32, tag="sB", bufs=2, name=f"winv_{wt}")
                nc.vector.reciprocal(out=winv[:], in_=wabs[:])
                winv2 = pool.tile([128, 1], F32, tag="sC", bufs=2, name=f"winv2_{wt}")
                nc.vector.tensor_scalar_mul(out=winv2[:], in0=winv[:], scalar1=224.0)
                wscale = pool.tile([128, 1], F32, tag="sD", bufs=2, name=f"wscale_{wt}")
                nc.vector.tensor_scalar_mul(out=wscale[:], in0=wabs[:], scalar1=INV224)
                nc.sync.dma_start(out=wsrow[0:1, wt * 128:(wt + 1) * 128],
                                  in_=wscale[:])
                wq = pool.tile([128, K], FP8, tag="wq8", bufs=1, name=f"wq_{wt}")
                nc.scalar.activation(out=wq[:], in_=wtile[:], func=ACTF.Copy,
                                     scale=winv2[:])
                nc.sync.dma_start_transpose(WqT[:, :, wt * 128:(wt + 1) * 128],
                                            wq[:].bitcast(BF16))

            xts = {}
            xqs = {}
            xqts = {}
            xshs = {}
            xnats = {}

            def x_pre(mt):
                xt = pool.tile([128, K], F32, tag="xt", bufs=2, name=f"xt_{mt}")
                nc.sync.dma_start(out=xt[:], in_=xs[mt * 128:(mt + 1) * 128])
                nc.scalar.activation(out=xt[:], in_=xt[:], func=ACTF.Relu)
                xts[mt] = xt

            def x_post(mt):
                r = xts[mt]
                r2 = r[:].rearrange("p (g two) -> p g two", two=2)
                pr = pool.tile([128, K // 2], F32, tag="pr", name=f"pr_{mt}")
                qs = pool.tile([128, K // 2], F32, tag="qs", name=f"qs_{mt}")
                nc.vector.tensor_tensor(out=pr[:], in0=r2[:, :, 0], in1=r2[:, :, 1], op=ALU.max)
                nc.vector.tensor_tensor(out=qs[:], in0=r2[:, :, 0], in1=r2[:, :, 1], op=ALU.min)

                rmax = pool.tile([128, 1], F32, tag="sE", bufs=2, name=f"rmax_{mt}")
                nc.vector.tensor_reduce(out=rmax[:], in_=pr[:],
                                        axis=mybir.AxisListType.X, op=ALU.max)

                pr2 = pr[:].rearrange("p (g two) -> p g two", two=2)
                qs2 = qs[:].rearrange("p (g two) -> p g two", two=2)
                u1t = pool.tile([128, K // 4], F32, tag="u1", name=f"u1_{mt}")
                u2t = pool.tile([128, K // 4], F32, tag="u2", name=f"u2_{mt}")
                thrt = pool.tile([128, K // 4], F32, tag="thr", name=f"thr_{mt}")
                nc.vector.tensor_tensor(out=u1t[:], in0=pr2[:, :, 0], in1=pr2[:, :, 1], op=ALU.min)
                nc.vector.tensor_tensor(out=u2t[:], in0=qs2[:, :, 0], in1=qs2[:, :, 1], op=ALU.max)
                nc.vector.tensor_tensor(out=thrt[:], in0=u1t[:], in1=u2t[:], op=ALU.max)
                thr = thrt[:]

                mask = pool.tile([128, K], FP8, tag="qs", name=f"mask_{mt}")
                r4 = r[:].rearrange("p (g four) -> p g four", four=4)
                m4 = mask[:].rearrange("p (g four) -> p g four", four=4)
                tb = thr.rearrange("p (g one) -> p g one", one=1)
                tb = dataclasses.replace(tb, ap=[tb.ap[0], tb.ap[1], [0, 4]])
                nc.vector.tensor_tensor(out=m4[:], in0=r4[:], in1=tb, op=ALU.is_ge)
                nc.vector.tensor_tensor(out=r[:], in0=r[:], in1=mask[:], op=ALU.mult)

                rm2 = pool.tile([128, 1], F32, tag="sF", bufs=2, name=f"rm2_{mt}")
                nc.vector.tensor_scalar_max(out=rm2[:], in0=rmax[:], scalar1=1e-5)
                rrec = pool.tile([128, 1], F32, tag="sG", bufs=2, name=f"rrec_{mt}")
                nc.vector.reciprocal(out=rrec[:], in_=rm2[:])
                sq = pool.tile([128, 1], F32, tag="sH", bufs=2, name=f"sq_{mt}")
                nc.vector.tensor_scalar_mul(out=sq[:], in0=rrec[:], scalar1=SQRT224)
                xsc = pool.tile([128, 1], F32, tag="sI", bufs=2, name=f"xsc_{mt}")
                nc.vector.tensor_tensor(out=xsc[:], in0=rmax[:], in1=rmax[:], op=ALU.mult)
                xsc2 = pool.tile([128, 1], F32, tag="sJ", bufs=2, name=f"xsc2_{mt}")
                nc.vector.tensor_scalar_mul(out=xsc2[:], in0=xsc[:], scalar1=INV224)
                xsh = pool.tile([128, 1], F32, tag="sK", bufs=2, name=f"xsh_{mt}")
                nc.vector.stream_shuffle(out=xsh[:], in_=xsc2[:], mask=rev32)
                xshs[mt] = xsh

                xq = pool.tile([128, K], FP8, tag="xq8", bufs=3, name=f"xq_{mt}")
                nc.scalar.activation(out=xq[:], in_=r[:], func=ACTF.Square, scale=sq[:])
                xqT = pool.tile([128, KB, 128], BF16, tag="xqT", bufs=4,
                                name=f"xqT_{mt}")
                nc.sync.dma_start_transpose(xqT[:], xq[:].bitcast(BF16))
                xqts[mt] = xqT
                xnat = pool.tile([128, 1], F32, tag="sL", bufs=4, name=f"xnat_{mt}")
                xsh = xshs.pop(mt)
                for q in range(4):
                    nc.sync.dma_start(out=xnat[32 * (3 - q):32 * (4 - q)],
                                      in_=xsh[32 * q:32 * (q + 1)])
                xnats[mt] = xnat

            def mm_mm(mt):
                xqT = xqts[mt]
                accs = [psp.tile([128, 512], F32, tag="acc", name=f"acc_{mt}_{ch}")
                        for ch in range(4)]
                wq8 = WqT[:].bitcast(FP8)  # [128, KB, 2*NS]
                xq8 = xqT[:].bitcast(FP8)  # [128, KB, 256]
                for ch in range(4):
                    for blk in range(KB):
                        lhs = xq8[:, blk, :]
                        rhs = wq8[:, blk, ch * 1024:(ch + 1) * 1024].rearrange(
                            "p (n two) -> p two n", two=2)
                        nc.tensor.matmul(accs[ch][:], lhs, rhs,
                                         start=(blk == 0), stop=(blk == KB - 1),
                                         perf_mode=mybir.MatmulPerfMode.DoubleRowSwInterleave)
                return accs

            def dequant(mt, accs):
                xnat = xnats[mt]
                ost = pool.tile([128, NS], BF16, tag="wq8", bufs=1, name=f"ost_{mt}")
                for ch in range(4):
                    nc.vector.scalar_tensor_tensor(
                        out=ost[:, ch * 512:(ch + 1) * 512],
                        in0=accs[ch][:], scalar=xnat[:],
                        in1=WscaleB[:, ch * 512:(ch + 1) * 512],
                        op0=ALU.mult, op1=ALU.mult)
                nc.sync.dma_start(out=out[mt * 128:(mt + 1) * 128], in_=ost[:])

            # ---- emission schedule (single DMA ring, software-pipelined) ----
            def bcast_all():
                # replicate wsrow [1, NS] across 128 partitions via a DRAM
                # doubling chain (PE-free, ring-only)
                nc.sync.dma_start(out=wsd[0:1], in_=wsrow[:])
                k = 1
                while k < 128:
                    nc.sync.dma_start(out=wsd[k:2 * k], in_=wsd[0:k])
                    k *= 2
                nc.sync.dma_start(out=WscaleB[:], in_=wsd[:])

            # W pipeline: loads run 2 ahead of the compute+transpose tail so the
            # single DMA ring never stalls a load behind a compute-gated
            # transpose.  x tiles 0/1 load early to warm the X pipeline.
            w_load(0)
            w_load(1)
            x_pre(0)
            x_pre(1)
            for wt in range(2, WT):
                w_load(wt)
                w_rest(wt - 2)
                if wt == 10:
                    x_post(0)
                    x_pre(2)
                if wt == 13:
                    x_post(1)
                    x_pre(3)
            w_rest(WT - 2)
            w_rest(WT - 1)
            bcast_all()
            x_post(2)
            x_pre(4)

            pending = {}
            for mt in range(NT):
                pending[mt] = mm_mm(mt)
                if mt + 3 < NT:
                    x_post(mt + 3)
                if mt + 5 < NT:
                    x_pre(mt + 5)
                if mt >= 1:
                    dequant(mt - 1, pending.pop(mt - 1))
            dequant(NT - 1, pending.pop(NT - 1))

    return nc


_cached_nc = None


def _get_nc():
    global _cached_nc
    if _cached_nc is None:
        _cached_nc = _build_program()
    return _cached_nc


def _run(x, W, trace=False):
    x = np.ascontiguousarray(x, dtype=np.float32)
    W = np.ascontiguousarray(W, dtype=np.float32)
    assert x.shape == (M, K) and W.shape == (NW, K)
    nc = _get_nc()
    in_maps = []
    for c in range(N_CORES):
        g, h = c % MG, c // MG
        xsh = x[g * MS:(g + 1) * MS].reshape(NT, 128, K)[:, ::-1, :].reshape(MS, K)
        in_maps.append({
            "xs": np.ascontiguousarray(xsh),
            "ws": W[h * NS:(h + 1) * NS],
        })
    res = run_bass_kernel_spmd(nc, in_maps, core_ids=list(range(N_CORES)),
                               trace=trace)
    outf = np.empty((M, NW), dtype=ml_dtypes.bfloat16)
    for c in range(N_CORES):
        g, h = c % MG, c // MG
        outf[g * MS:(g + 1) * MS, h * NS:(h + 1) * NS] = res.results[c]["out"]
    return outf, res


def kernel(x, W):
    out, _ = _run(x, W, trace=False)
    return out

